# revision 5
# baseline (speedup 1.0000x reference)
"""Multi-head attention + output projection on 8 Trainium2 NeuronCores.

Problem (hardcoded): B=2, N=S=2048, DIM=1024, 8 heads, head_dim=128, fp32.
  out = softmax(Q K^T / sqrt(128)) V  -> reshape -> @ proj_w.T + proj_b

Sharding: data parallel on batch (2) x tensor parallel on heads (4 groups of
2 heads).  Each core computes attention for its 2 heads plus the partial
output projection restricted to its heads' columns; the host sums the 4
partial projections per batch and adds the bias.

Per-core kernel (matmul operands fp16, accumulation fp32 PSUM):
  S^T = K @ Q^T per 128-row s-chunk with s on partitions (softmax needs no
  on-chip transpose of P); exp on ScalarE (PSUM->SBUF, scale pre-applied to
  Q on host); out^T = V^T @ expS^T accumulated in PSUM.  Row sums feed an
  all-ones [128x128] matmul that colsums over partitions with the result
  broadcast to all 128 rows; reciprocal_approx_fast + multiply normalizes.

  The emission order is software-pipelined for the in-order engine queues:
  PV lags four groups behind QK/exp (hiding cross-engine semaphore
  latency), and each head's tail work (last PV pairs, rowsum matmuls,
  reciprocal, normalize) plus each block's projection are deferred into
  the NEXT head's stream so the exp pipeline on ScalarE never drains at
  head boundaries.

v2 changes over the first working version:
  - All inputs live in ONE host-packed dram tensor (partition-major,
    ordered by first use) mirrored 1:1 by one SBUF mega-tile, loaded by
    fine-grained contiguous DMAs so the first QK depends on a single small
    DMA instead of the whole input set.
  - ~20 dummy warm-up matmuls run during the input DMA window so the PE
    HAM clock gate is (mostly) released before the first real matmul.
  - Rowsum fold adds use scalar_tensor_tensor (TensorScalarPtr supports
    the DVE 4x perf mode; TensorTensor only gets 2x).
  - Projection PSUM->SBUF copies moved from VectorE to the idle GpSimd
    engine, casting to fp16; output DMAs are fp16 (host sums partials in
    fp32) and issued per 128-row chunk right after each copy.
"""

import sys
from collections import deque

sys.path.insert(0, "/opt/trn_rl_repo")

import numpy as np

import concourse.bass as bass  # noqa: F401  (engine namespaces live on nc)
import concourse.mybir as mybir
import concourse.tile as tile
from concourse import bacc
from concourse.bass_utils import run_bass_kernel_spmd

B = 2
N = 2048
S = 2048
DIM = 1024
NUM_HEADS = 8
HD = 128
N_CORES = 8
HEADS_PER_CORE = 2  # 4-way head parallel x 2-way batch parallel
HG = DIM // (NUM_HEADS // HEADS_PER_CORE)  # 256 dims per core
P = 128
SC = S // P  # 16 s-chunks
NB = 512  # query-column block
NQ = N // NB
GC = 2  # s-chunks per exp group
NG = SC // GC  # 8 groups per (head, block)
F32 = mybir.dt.float32
F16 = mybir.dt.float16

WARM_MMS = 20  # dummy matmuls to release the PE HAM clock gate during DMA-in

# ---- packed input layout: column offsets (fp16 elements per partition) ----
# ordered by first use inside the kernel
OFF_KT0_LO = 0  # kt h0, s 0:1024
OFF_QT0_B0 = 1024  # qt h0, n 0:512
OFF_KT0_HI = 1536  # kt h0, s 1024:2048
OFF_V0_LO = 2560  # v  h0, chunks 0:8   (8*128 d-major)
OFF_QT1_B0 = 3584  # qt h1, n 0:512
OFF_KT1 = 4096  # kt h1, s 0:2048
OFF_V0_HI = 6144  # v  h0, chunks 8:16
OFF_V1 = 7168  # v  h1, chunks 0:16
OFF_QT0_REST = 9216  # qt h0, n 512:2048
OFF_QT1_REST = 10752  # qt h1, n 512:2048
OFF_WT = 12288  # wt h0 (1024) | wt h1 (1024)
PACK_COLS = 14336


def _kt_off(h, si):
    if h == 1:
        return OFF_KT1 + si * P
    return (OFF_KT0_LO + si * P) if si < 8 else (OFF_KT0_HI + (si - 8) * P)


def _qt_off(h, nq):
    if nq == 0:
        return OFF_QT0_B0 if h == 0 else OFF_QT1_B0
    base = OFF_QT0_REST if h == 0 else OFF_QT1_REST
    return base + (nq - 1) * NB


def _v_off(h, c):
    if h == 1:
        return OFF_V1 + c * HD
    return (OFF_V0_LO + c * HD) if c < 8 else (OFF_V0_HI + (c - 8) * HD)


def _wt_off(h, o):
    return OFF_WT + h * DIM + o


_nc_cache = {}


def _build():
    nc = bacc.Bacc(None, target_bir_lowering=False, debug=False, num_devices=1)

    inp = nc.dram_tensor("inp", [P, PACK_COLS], F16, kind="ExternalInput").ap()
    out = nc.dram_tensor("out", [P, N // P, DIM], F16, kind="ExternalOutput").ap()

    EXPF = mybir.ActivationFunctionType.Exp
    ADD = mybir.AluOpType.add
    MULT = mybir.AluOpType.mult

    with tile.TileContext(nc) as tc:
        with (
            tc.tile_pool(name="persist", bufs=1) as persist,
            tc.tile_pool(name="e_pool", bufs=10) as e_pool,
            tc.tile_pool(name="a_pool", bufs=3) as a_pool,
            tc.tile_pool(name="small", bufs=3) as small,
            tc.tile_pool(name="y_pool", bufs=2) as y_pool,
            tc.tile_pool(name="s_ps_pool", bufs=2, space="PSUM") as s_ps_pool,
            tc.tile_pool(name="acc_ps_pool", bufs=4, space="PSUM") as acc_ps_pool,
        ):
            # One resident SBUF tile mirrors the packed dram layout 1:1 so
            # each DMA is a contiguous [128, w] slice copy and the first QK
            # group depends only on the first small DMA.
            kq_sb = persist.tile([P, PACK_COLS], F16)
            ones_dram = nc.inline_tensor(np.ones((P, P), np.float16), name="ones_const")
            ones_mat = persist.tile([P, P], F16)
            warm_sb = persist.tile([P, P], F16)

            def ld(a, b):
                nc.sync.dma_start(out=kq_sb[:, a:b], in_=inp[:, a:b])

            # PE warm-up: garbage matmuls into a PSUM tile that the first
            # real PV accumulation later overwrites with start=True.
            nc.vector.memset(warm_sb, 0.0)
            warm_ps = acc_ps_pool.tile([P, NB], F32, tag="acc")
            for w in range(WARM_MMS):
                nc.tensor.matmul(
                    warm_ps[:, 0:P], warm_sb, warm_sb, start=True, stop=True
                )

            ld(OFF_KT0_LO, OFF_KT0_HI)  # kt h0 lo + qt h0 b0 (one DMA)
            ld(OFF_KT0_HI, OFF_V0_LO)  # kt h0 hi
            ld(OFF_V0_LO, OFF_QT1_B0)  # v h0 lo
            nc.sync.dma_start(out=ones_mat, in_=ones_dram.ap())
            ld(OFF_QT1_B0, OFF_V0_HI)  # qt h1 b0 + kt h1 (one DMA)
            ld(OFF_V0_HI, OFF_V1)  # v h0 hi
            ld(OFF_V1, OFF_QT0_REST)  # v h1
            ld(OFF_QT0_REST, OFF_QT1_REST)  # qt h0 rest
            ld(OFF_QT1_REST, OFF_WT)  # qt h1 rest
            ld(OFF_WT, PACK_COLS)  # wt

            # X^T: normalized attention outputs, head-dim on partitions.
            xt_sb = persist.tile([P, HEADS_PER_CORE, N], F16)

            def pv_pair(o_ps, h, e_t, g):
                def fn():
                    for j in range(GC):
                        si = GC * g + j
                        nc.tensor.matmul(
                            o_ps,
                            kq_sb[:, _v_off(h, si) : _v_off(h, si) + HD],
                            e_t[:, j, :],
                            start=False,
                            stop=(si == SC - 1),
                        )

                return fn

            def fold(acc, ea, eb):
                # acc = ea + eb elementwise; TensorScalarPtr gets the DVE 4x
                # perf mode (all-SBUF fp16), TensorTensor only 2x.
                with nc.allow_low_precision(reason="fp16 rowsum partials"):
                    nc.vector.scalar_tensor_tensor(
                        out=acc, in0=ea, scalar=1.0, in1=eb, op0=MULT, op1=ADD
                    )

            def adds_pair(a2, a2g, ea, eb):
                def fn():
                    fold(a2, a2, ea)
                    fold(a2g, a2g, eb)

                return fn

            def finish_vh(o_ps, rb_ps, a2, a2g, h, nsl):
                def fn():
                    a1 = small.tile([P, NB], F16, tag="a1")
                    fold(a2, a2, a2g)
                    fold(a1, a2[:, 0, :], a2[:, 1, :])
                    nc.tensor.matmul(rb_ps, ones_mat, a1, start=True, stop=True)
                    recip = small.tile([P, NB], F32, tag="recip")
                    nc.vector.reciprocal_approx_fast(out=recip, in_=rb_ps)
                    with nc.allow_low_precision(reason="fp16 attention output grid"):
                        nc.vector.tensor_mul(xt_sb[:, h, nsl], o_ps, recip)

                return fn

            def proj_half(nq, t, ot, y_sb):
                def fn():
                    nt = nq * (NB // P) + t
                    y_ps = acc_ps_pool.tile([P, NB], F32, tag="acc")
                    for hh in range(HEADS_PER_CORE):
                        wo = _wt_off(hh, ot * NB)
                        nc.tensor.matmul(
                            y_ps,
                            xt_sb[:, hh, nt * P : (nt + 1) * P],
                            kq_sb[:, wo : wo + NB],
                            start=(hh == 0),
                            stop=(hh == HEADS_PER_CORE - 1),
                        )
                    with nc.allow_low_precision(reason="fp16 partial projection"):
                        nc.vector.tensor_copy(
                            y_sb[:, t, ot * NB : (ot + 1) * NB], y_ps
                        )
                    if ot == 1:
                        nc.sync.dma_start(
                            out=out[:, nq * (NB // P) + t, :], in_=y_sb[:, t, :]
                        )

                return fn

            work_q = deque()
            for nq in range(NQ):
                nsl = slice(nq * NB, (nq + 1) * NB)
                for h in range(HEADS_PER_CORE):
                    qo = _qt_off(h, nq)
                    q_blk = kq_sb[:, qo : qo + NB]
                    o_ps = acc_ps_pool.tile([P, NB], F32, tag="acc")
                    a2 = a_pool.tile([P, GC, NB], F16, tag="a2")
                    a2g = a_pool.tile([P, GC, NB], F16, tag="a2g")
                    rb_ps = None
                    es = []  # exp tiles in flight
                    for g in range(NG):
                        s_ps = s_ps_pool.tile([P, GC, NB], F32, tag="s")
                        for j in range(GC):
                            si = GC * g + j
                            ko = _kt_off(h, si)
                            nc.tensor.matmul(
                                s_ps[:, j, :],
                                kq_sb[:, ko : ko + P],
                                q_blk,
                                start=True,
                                stop=True,
                            )
                        e_t = e_pool.tile([P, GC, NB], F16, tag="e")
                        nc.scalar.activation(out=e_t, in_=s_ps, func=EXPF)
                        es.append(e_t)

                        # drain deferred work: one small closure per group,
                        # two when the queue runs deep
                        if work_q:
                            work_q.popleft()()
                            if len(work_q) >= 7:
                                work_q.popleft()()

                        # PV + rowsum accumulation lag four groups behind exp
                        if g >= 4:
                            pg = g - 4
                            pe = es[pg]
                            for j in range(GC):
                                si = GC * pg + j
                                nc.tensor.matmul(
                                    o_ps,
                                    kq_sb[:, _v_off(h, si) : _v_off(h, si) + HD],
                                    pe[:, j, :],
                                    start=(si == 0),
                                    stop=False,
                                )
                            if pg in (2, 3):
                                acc = a2 if pg == 2 else a2g
                                fold(acc, es[pg - 2], pe)
                        if g == NG - 1:
                            rb_ps = acc_ps_pool.tile([P, NB], F32, tag="acc")
                    # defer PV of groups 4..7, remaining folds, and normalize
                    work_q.append(pv_pair(o_ps, h, es[4], 4))
                    work_q.append(adds_pair(a2, a2g, es[4], es[5]))
                    work_q.append(pv_pair(o_ps, h, es[5], 5))
                    work_q.append(pv_pair(o_ps, h, es[6], 6))
                    work_q.append(adds_pair(a2, a2g, es[6], es[7]))
                    work_q.append(pv_pair(o_ps, h, es[7], 7))
                    work_q.append(finish_vh(o_ps, rb_ps, a2, a2g, h, nsl))
                y_sb = y_pool.tile([P, NB // P, DIM], F16, tag="y")
                for t in range(NB // P):
                    for ot in range(2):
                        work_q.append(proj_half(nq, t, ot, y_sb))

            while work_q:
                work_q.popleft()()

    nc.compile()
    return nc


def make_in_maps(query, key, value, proj_w):
    scale = float(HD) ** -0.5
    wt_full = np.ascontiguousarray(proj_w.T.astype(np.float32))  # [in, out]
    in_maps = []
    for core in range(N_CORES):
        b, hg = divmod(core, N_CORES // B)
        sl = slice(hg * HG, (hg + 1) * HG)
        # per-head partition-major views: [h][p][...]
        qt = (query[b].T[sl] * scale).astype(np.float16).reshape(2, P, N)
        kt = key[b].T[sl].astype(np.float16).reshape(2, P, S)
        # v[p, h, c, d] = value[c*128+p, h*128+d] -> [h][c][p][d]
        vv = (
            value[b][:, sl]
            .astype(np.float16)
            .reshape(SC, P, HEADS_PER_CORE, HD)
            .transpose(2, 0, 1, 3)
        )
        wt = wt_full[sl].astype(np.float16).reshape(2, P, DIM)

        pack = np.empty((P, PACK_COLS), dtype=np.float16)
        pack[:, OFF_KT0_LO:OFF_QT0_B0] = kt[0, :, 0:1024]
        pack[:, OFF_QT0_B0:OFF_KT0_HI] = qt[0, :, 0:NB]
        pack[:, OFF_KT0_HI:OFF_V0_LO] = kt[0, :, 1024:2048]
        pack[:, OFF_V0_LO:OFF_QT1_B0] = (
            vv[0, 0:8].transpose(1, 0, 2).reshape(P, 8 * HD)
        )
        pack[:, OFF_QT1_B0:OFF_KT1] = qt[1, :, 0:NB]
        pack[:, OFF_KT1:OFF_V0_HI] = kt[1]
        pack[:, OFF_V0_HI:OFF_V1] = vv[0, 8:16].transpose(1, 0, 2).reshape(P, 8 * HD)
        pack[:, OFF_V1:OFF_QT0_REST] = (
            vv[1].transpose(1, 0, 2).reshape(P, SC * HD)
        )
        pack[:, OFF_QT0_REST:OFF_QT1_REST] = qt[0, :, NB:]
        pack[:, OFF_QT1_REST:OFF_WT] = qt[1, :, NB:]
        pack[:, OFF_WT : OFF_WT + DIM] = wt[0]
        pack[:, OFF_WT + DIM :] = wt[1]
        in_maps.append({"inp": pack})
    return in_maps


def kernel(query, key, value, proj_w, proj_b):
    query = np.asarray(query)
    key = np.asarray(key)
    value = np.asarray(value)
    proj_w = np.asarray(proj_w)
    proj_b = np.asarray(proj_b)
    if "nc" not in _nc_cache:
        _nc_cache["nc"] = _build()
    nc = _nc_cache["nc"]

    in_maps = make_in_maps(query, key, value, proj_w)
    res = run_bass_kernel_spmd(nc, in_maps, list(range(N_CORES)))

    out = np.zeros((B, N, DIM), dtype=np.float32)
    for core in range(N_CORES):
        b = core // (N_CORES // B)
        # out dram is [p, chunk, o] with row n = chunk*128 + p
        part = res.results[core]["out"]
        out[b] += part.transpose(1, 0, 2).reshape(N, DIM).astype(np.float32)
    out += proj_b.astype(np.float32)
    return out


# revision 13
# speedup vs baseline: 1.1466x; 1.1466x over previous
"""Multi-head attention + output projection on 8 Trainium2 NeuronCores.

Problem (hardcoded): B=2, N=S=2048, DIM=1024, 8 heads, head_dim=128, fp32.
  out = softmax(Q K^T / sqrt(128)) V  -> reshape -> @ proj_w.T + proj_b

Sharding: data parallel on batch (2) x tensor parallel on heads (4 groups of
2 heads).  Each core computes attention for its 2 heads plus the partial
output projection restricted to its heads' columns; the host sums the 4
partial projections per batch and adds the bias.

Per-core kernel (matmul operands fp16, accumulation fp32 PSUM):
  S^T = K @ Q^T per 128-row s-chunk with s on partitions (softmax needs no
  on-chip transpose of P); exp on ScalarE (PSUM->SBUF, scale pre-applied to
  Q on host); out^T = V^T @ expS^T accumulated in PSUM.  Row sums feed an
  all-ones [128x128] matmul that colsums over partitions with the result
  broadcast to all 128 rows; reciprocal_approx_fast + multiply normalizes.

  The emission order is software-pipelined for the in-order engine queues:
  PV lags four groups behind QK/exp (hiding cross-engine semaphore
  latency), and each head's tail work (last PV pairs, rowsum matmuls,
  reciprocal, normalize) plus each block's projection are deferred into
  the NEXT head's stream so the exp pipeline on ScalarE never drains at
  head boundaries.

v2 changes over the first working version:
  - All inputs live in ONE host-packed dram tensor (partition-major,
    ordered by first use) mirrored 1:1 by one SBUF mega-tile, loaded by
    fine-grained contiguous DMAs so the first QK depends on a single small
    DMA instead of the whole input set.
  - ~20 dummy warm-up matmuls run during the input DMA window so the PE
    HAM clock gate is (mostly) released before the first real matmul.
  - Rowsum fold adds use scalar_tensor_tensor (TensorScalarPtr supports
    the DVE 4x perf mode; TensorTensor only gets 2x).
  - Projection PSUM->SBUF copies moved from VectorE to the idle GpSimd
    engine, casting to fp16; output DMAs are fp16 (host sums partials in
    fp32) and issued per 128-row chunk right after each copy.
"""

import sys
from collections import deque

sys.path.insert(0, "/opt/trn_rl_repo")

import numpy as np

import concourse.bass as bass  # noqa: F401  (engine namespaces live on nc)
import concourse.mybir as mybir
import concourse.tile as tile
from concourse import bacc
from concourse.bass_utils import run_bass_kernel_spmd

B = 2
N = 2048
S = 2048
DIM = 1024
NUM_HEADS = 8
HD = 128
N_CORES = 8
HEADS_PER_CORE = 2  # 4-way head parallel x 2-way batch parallel
HG = DIM // (NUM_HEADS // HEADS_PER_CORE)  # 256 dims per core
P = 128
SC = S // P  # 16 s-chunks
NB = 512  # query-column block
NQ = N // NB
GC = 2  # s-chunks per exp group
NG = SC // GC  # 8 groups per (head, block)
F32 = mybir.dt.float32
F16 = mybir.dt.float16

WARM_MMS = 14  # dummy matmuls to release the PE HAM clock gate during DMA-in
# (sized to end right when the first input DMA lands, ~8.4us; more would
# delay the first real matmul behind the warm-up stream)

# ---- packed input layout: column offsets (fp16 elements per partition) ----
# ordered by first use inside the kernel
OFF_QT0_B0 = 0  # qt h0, n 0:512
OFF_KT0 = 512  # kt h0, s 0:2048
OFF_V0_LO = 2560  # v  h0, chunks 0:8   (8*128 d-major)
OFF_QT1_B0 = 3584  # qt h1, n 0:512
OFF_KT1 = 4096  # kt h1, s 0:2048
OFF_V0_HI = 6144  # v  h0, chunks 8:16
OFF_V1 = 7168  # v  h1, chunks 0:16
OFF_QT0_REST = 9216  # qt h0, n 512:2048
OFF_QT1_REST = 10752  # qt h1, n 512:2048
OFF_WT = 12288  # wt h0 (1024) | wt h1 (1024)
PACK_COLS = 14336


def _kt_off(h, si):
    return (OFF_KT1 if h == 1 else OFF_KT0) + si * P


def _qt_off(h, nq):
    if nq == 0:
        return OFF_QT0_B0 if h == 0 else OFF_QT1_B0
    base = OFF_QT0_REST if h == 0 else OFF_QT1_REST
    return base + (nq - 1) * NB


def _v_off(h, c):
    if h == 1:
        return OFF_V1 + c * HD
    return (OFF_V0_LO + c * HD) if c < 8 else (OFF_V0_HI + (c - 8) * HD)


def _wt_off(h, o):
    return OFF_WT + h * DIM + o


_nc_cache = {}


def _build():
    nc = bacc.Bacc(None, target_bir_lowering=False, debug=False, num_devices=1)

    inp = nc.dram_tensor("inp", [P, PACK_COLS], F16, kind="ExternalInput").ap()
    out = nc.dram_tensor("out", [P, N // P, DIM], F16, kind="ExternalOutput").ap()

    EXPF = mybir.ActivationFunctionType.Exp
    ADD = mybir.AluOpType.add
    MULT = mybir.AluOpType.mult

    with tile.TileContext(nc) as tc:
        with (
            tc.tile_pool(name="persist", bufs=1) as persist,
            tc.tile_pool(name="e_pool", bufs=10) as e_pool,
            tc.tile_pool(name="a_pool", bufs=3) as a_pool,
            tc.tile_pool(name="small", bufs=3) as small,
            tc.tile_pool(name="y_pool", bufs=2) as y_pool,
            tc.tile_pool(name="s_ps_pool", bufs=2, space="PSUM") as s_ps_pool,
            tc.tile_pool(name="acc_ps_pool", bufs=4, space="PSUM") as acc_ps_pool,
        ):
            # One resident SBUF tile mirrors the packed dram layout 1:1 so
            # each DMA is a contiguous [128, w] slice copy and the first QK
            # group depends only on the first small DMA.
            kq_sb = persist.tile([P, PACK_COLS], F16)
            ones_dram = nc.inline_tensor(np.ones((P, P), np.float16), name="ones_const")
            ones_mat = persist.tile([P, P], F16)
            warm_sb = persist.tile([P, P], F16)

            def ld(a, b):
                nc.sync.dma_start(out=kq_sb[:, a:b], in_=inp[:, a:b])

            # PE warm-up: garbage matmuls into a PSUM tile that the first
            # real PV accumulation later overwrites with start=True.
            nc.vector.memset(warm_sb, 0.0)
            warm_ps = acc_ps_pool.tile([P, NB], F32, tag="acc")
            for w in range(WARM_MMS):
                nc.tensor.matmul(
                    warm_ps[:, 0:P], warm_sb, warm_sb, start=True, stop=True
                )

            ld(OFF_QT0_B0, OFF_KT0 + 512)  # qt h0 b0 + kt h0 chunks 0-3
            ld(OFF_KT0 + 512, OFF_V0_LO)  # kt h0 chunks 4-15
            ld(OFF_V0_LO, OFF_QT1_B0)  # v h0 lo
            nc.sync.dma_start(out=ones_mat, in_=ones_dram.ap())
            ld(OFF_QT1_B0, OFF_V0_HI)  # qt h1 b0 + kt h1 (one DMA)
            ld(OFF_V0_HI, OFF_V1)  # v h0 hi
            ld(OFF_V1, OFF_QT0_REST)  # v h1
            ld(OFF_QT0_REST, OFF_QT1_REST)  # qt h0 rest
            ld(OFF_QT1_REST, OFF_WT)  # qt h1 rest
            ld(OFF_WT, PACK_COLS)  # wt

            # X^T: normalized attention outputs, head-dim on partitions.
            xt_sb = persist.tile([P, HEADS_PER_CORE, N], F16)

            def pv_pair(o_ps, h, e_t, g):
                def fn():
                    for j in range(GC):
                        si = GC * g + j
                        nc.tensor.matmul(
                            o_ps,
                            kq_sb[:, _v_off(h, si) : _v_off(h, si) + HD],
                            e_t[:, j, :],
                            start=False,
                            stop=(si == SC - 1),
                        )

                return fn

            def fold(acc, ea, eb, eng=None):
                # acc = ea + eb elementwise fp16 (DVE runs this in 2x mode);
                # some deferred folds go to the otherwise-idle GpSimd.
                with nc.allow_low_precision(reason="fp16 rowsum partials"):
                    (eng or nc.vector).tensor_add(acc, ea, eb)

            def adds_pair(a2, a2g, ea, eb, eng=None):
                def fn():
                    fold(a2, a2, ea, eng)
                    fold(a2g, a2g, eb, eng)

                return fn

            def finish_vh(o_ps, rb_ps, a2, a2g, h, nsl):
                def fn():
                    a1 = small.tile([P, NB], F16, tag="a1")
                    fold(a2, a2, a2g)
                    fold(a1, a2[:, 0, :], a2[:, 1, :])
                    nc.tensor.matmul(rb_ps, ones_mat, a1, start=True, stop=True)
                    recip = small.tile([P, NB], F32, tag="recip")
                    nc.vector.reciprocal_approx_fast(out=recip, in_=rb_ps)
                    with nc.allow_low_precision(reason="fp16 attention output grid"):
                        nc.vector.tensor_mul(xt_sb[:, h, nsl], o_ps, recip)

                return fn

            def proj_half(nq, t, ot, y_sb, on_scalar=False):
                def fn():
                    nt = nq * (NB // P) + t
                    y_ps = acc_ps_pool.tile([P, NB], F32, tag="acc")
                    for hh in range(HEADS_PER_CORE):
                        wo = _wt_off(hh, ot * NB)
                        nc.tensor.matmul(
                            y_ps,
                            xt_sb[:, hh, nt * P : (nt + 1) * P],
                            kq_sb[:, wo : wo + NB],
                            start=(hh == 0),
                            stop=(hh == HEADS_PER_CORE - 1),
                        )
                    dst = y_sb[:, t, ot * NB : (ot + 1) * NB]
                    with nc.allow_low_precision(reason="fp16 partial projection"):
                        if on_scalar:
                            # last block only: ScalarE's exp stream is done
                            nc.scalar.activation(
                                out=dst, in_=y_ps,
                                func=mybir.ActivationFunctionType.Copy,
                            )
                        else:
                            nc.vector.tensor_copy(dst, y_ps)
                    if ot == 1:
                        nc.sync.dma_start(
                            out=out[:, nq * (NB // P) + t, :], in_=y_sb[:, t, :]
                        )

                return fn

            work_q = deque()
            for nq in range(NQ):
                nsl = slice(nq * NB, (nq + 1) * NB)
                for h in range(HEADS_PER_CORE):
                    last_vh = nq == NQ - 1 and h == HEADS_PER_CORE - 1
                    lag = 2 if last_vh else 4
                    qo = _qt_off(h, nq)
                    q_blk = kq_sb[:, qo : qo + NB]
                    o_ps = acc_ps_pool.tile([P, NB], F32, tag="acc")
                    a2 = a_pool.tile([P, GC, NB], F16, tag="a2")
                    a2g = a_pool.tile([P, GC, NB], F16, tag="a2g")
                    rb_ps = None
                    es = []  # exp tiles in flight
                    for g in range(NG):
                        s_ps = s_ps_pool.tile([P, GC, NB], F32, tag="s")
                        for j in range(GC):
                            si = GC * g + j
                            ko = _kt_off(h, si)
                            nc.tensor.matmul(
                                s_ps[:, j, :],
                                kq_sb[:, ko : ko + P],
                                q_blk,
                                start=True,
                                stop=True,
                            )
                        e_t = e_pool.tile([P, GC, NB], F16, tag="e")
                        nc.scalar.activation(out=e_t, in_=s_ps, func=EXPF)
                        es.append(e_t)

                        # drain deferred work: one small closure per group,
                        # two when the queue runs deep
                        if work_q:
                            work_q.popleft()()
                            if len(work_q) >= 7 and work_q:
                                work_q.popleft()()

                        # PV + rowsum accumulation lag behind exp
                        if g >= lag:
                            pg = g - lag
                            pe = es[pg]
                            for j in range(GC):
                                si = GC * pg + j
                                nc.tensor.matmul(
                                    o_ps,
                                    kq_sb[:, _v_off(h, si) : _v_off(h, si) + HD],
                                    pe[:, j, :],
                                    start=(si == 0),
                                    stop=False,
                                )
                            if pg == 2:
                                fold(a2, es[0], pe)
                            elif pg == 3:
                                fold(a2g, es[1], pe)
                            elif pg == 4:  # lag==2 only
                                fold(a2, a2, pe)
                            elif pg == 5:  # lag==2 only
                                fold(a2g, a2g, pe)
                        if g == NG - 1:
                            rb_ps = acc_ps_pool.tile([P, NB], F32, tag="acc")
                    # defer PV of the trailing groups, remaining folds, and
                    # normalize; the first deferred fold pair runs on GpSimd
                    for dg in range(NG - lag, NG):
                        if dg == 4:
                            work_q.append(
                                adds_pair(a2, a2g, es[4], es[5], nc.gpsimd)
                            )
                        if dg == 6:
                            work_q.append(adds_pair(a2, a2g, es[6], es[7]))
                        work_q.append(pv_pair(o_ps, h, es[dg], dg))
                    work_q.append(finish_vh(o_ps, rb_ps, a2, a2g, h, nsl))
                y_sb = y_pool.tile([P, NB // P, DIM], F16, tag="y")
                for t in range(NB // P):
                    for ot in range(2):
                        work_q.append(
                            proj_half(nq, t, ot, y_sb, on_scalar=(nq == NQ - 1))
                        )

            while work_q:
                work_q.popleft()()

    nc.compile()
    return nc


def make_in_maps(query, key, value, proj_w):
    scale = float(HD) ** -0.5
    wt_full = np.ascontiguousarray(proj_w.T.astype(np.float32))  # [in, out]
    in_maps = []
    for core in range(N_CORES):
        b, hg = divmod(core, N_CORES // B)
        sl = slice(hg * HG, (hg + 1) * HG)
        # per-head partition-major views: [h][p][...]
        qt = (query[b].T[sl] * scale).astype(np.float16).reshape(2, P, N)
        kt = key[b].T[sl].astype(np.float16).reshape(2, P, S)
        # v[p, h, c, d] = value[c*128+p, h*128+d] -> [h][c][p][d]
        vv = (
            value[b][:, sl]
            .astype(np.float16)
            .reshape(SC, P, HEADS_PER_CORE, HD)
            .transpose(2, 0, 1, 3)
        )
        wt = wt_full[sl].astype(np.float16).reshape(2, P, DIM)

        pack = np.empty((P, PACK_COLS), dtype=np.float16)
        pack[:, OFF_QT0_B0:OFF_KT0] = qt[0, :, 0:NB]
        pack[:, OFF_KT0:OFF_V0_LO] = kt[0]
        pack[:, OFF_V0_LO:OFF_QT1_B0] = (
            vv[0, 0:8].transpose(1, 0, 2).reshape(P, 8 * HD)
        )
        pack[:, OFF_QT1_B0:OFF_KT1] = qt[1, :, 0:NB]
        pack[:, OFF_KT1:OFF_V0_HI] = kt[1]
        pack[:, OFF_V0_HI:OFF_V1] = vv[0, 8:16].transpose(1, 0, 2).reshape(P, 8 * HD)
        pack[:, OFF_V1:OFF_QT0_REST] = (
            vv[1].transpose(1, 0, 2).reshape(P, SC * HD)
        )
        pack[:, OFF_QT0_REST:OFF_QT1_REST] = qt[0, :, NB:]
        pack[:, OFF_QT1_REST:OFF_WT] = qt[1, :, NB:]
        pack[:, OFF_WT : OFF_WT + DIM] = wt[0]
        pack[:, OFF_WT + DIM :] = wt[1]
        in_maps.append({"inp": pack})
    return in_maps


def kernel(query, key, value, proj_w, proj_b):
    query = np.asarray(query)
    key = np.asarray(key)
    value = np.asarray(value)
    proj_w = np.asarray(proj_w)
    proj_b = np.asarray(proj_b)
    if "nc" not in _nc_cache:
        _nc_cache["nc"] = _build()
    nc = _nc_cache["nc"]

    in_maps = make_in_maps(query, key, value, proj_w)
    res = run_bass_kernel_spmd(nc, in_maps, list(range(N_CORES)))

    out = np.zeros((B, N, DIM), dtype=np.float32)
    for core in range(N_CORES):
        b = core // (N_CORES // B)
        # out dram is [p, chunk, o] with row n = chunk*128 + p
        part = res.results[core]["out"]
        out[b] += part.transpose(1, 0, 2).reshape(N, DIM).astype(np.float32)
    out += proj_b.astype(np.float32)
    return out


# revision 18
# speedup vs baseline: 1.2084x; 1.0539x over previous
"""Multi-head attention + output projection on 8 Trainium2 NeuronCores.

Problem (hardcoded): B=2, N=S=2048, DIM=1024, 8 heads, head_dim=128, fp32.
  out = softmax(Q K^T / sqrt(128)) V  -> reshape -> @ proj_w.T + proj_b

Sharding: data parallel on batch (2) x tensor parallel on heads (4 groups of
2 heads).  Each core computes attention for its 2 heads plus the partial
output projection restricted to its heads' columns; the host sums the 4
partial projections per batch and adds the bias.

Per-core kernel (matmul operands fp16, accumulation fp32 PSUM):
  S^T = K @ Q^T per 128-row s-chunk with s on partitions (softmax needs no
  on-chip transpose of P); exp on ScalarE (PSUM->SBUF, scale pre-applied to
  Q on host); out^T = V^T @ expS^T accumulated in PSUM.  Row sums feed an
  all-ones [128x128] matmul that colsums over partitions with the result
  broadcast to all 128 rows; reciprocal_approx_fast + multiply normalizes.

  The emission order is software-pipelined for the in-order engine queues:
  PV lags four groups behind QK/exp (hiding cross-engine semaphore
  latency), and each head's tail work (last PV pairs, rowsum matmuls,
  reciprocal, normalize) plus each block's projection are deferred into
  the NEXT head's stream so the exp pipeline on ScalarE never drains at
  head boundaries.

v2 changes over the first working version:
  - All inputs live in ONE host-packed dram tensor (partition-major,
    ordered by first use) mirrored 1:1 by one SBUF mega-tile, loaded by
    fine-grained contiguous DMAs so the first QK depends on a single small
    DMA instead of the whole input set.
  - ~20 dummy warm-up matmuls run during the input DMA window so the PE
    HAM clock gate is (mostly) released before the first real matmul.
  - Rowsum fold adds use scalar_tensor_tensor (TensorScalarPtr supports
    the DVE 4x perf mode; TensorTensor only gets 2x).
  - Projection PSUM->SBUF copies moved from VectorE to the idle GpSimd
    engine, casting to fp16; output DMAs are fp16 (host sums partials in
    fp32) and issued per 128-row chunk right after each copy.
"""

import sys
from collections import deque

sys.path.insert(0, "/opt/trn_rl_repo")

import numpy as np

import concourse.bass as bass  # noqa: F401  (engine namespaces live on nc)
import concourse.mybir as mybir
import concourse.tile as tile
from concourse import bacc
from concourse.bass_utils import run_bass_kernel_spmd

B = 2
N = 2048
S = 2048
DIM = 1024
NUM_HEADS = 8
HD = 128
N_CORES = 8
HEADS_PER_CORE = 2  # 4-way head parallel x 2-way batch parallel
HG = DIM // (NUM_HEADS // HEADS_PER_CORE)  # 256 dims per core
P = 128
SC = S // P  # 16 s-chunks
NB = 512  # query-column block
NQ = N // NB
GC = 2  # s-chunks per exp group
NG = SC // GC  # 8 groups per (head, block)
F32 = mybir.dt.float32
F16 = mybir.dt.float16

WARM_MMS = 14  # dummy matmuls to release the PE HAM clock gate during DMA-in
# (sized to end right when the first input DMA lands, ~8.4us; more would
# delay the first real matmul behind the warm-up stream)

# ---- packed input layout: column offsets (fp16 elements per partition) ----
# ordered by first use inside the kernel
OFF_QT0_B0 = 0  # qt h0, n 0:512
OFF_KT0 = 512  # kt h0, s 0:2048
OFF_V0_LO = 2560  # v  h0, chunks 0:8   (8*128 d-major)
OFF_QT1_B0 = 3584  # qt h1, n 0:512
OFF_KT1 = 4096  # kt h1, s 0:2048
OFF_V0_HI = 6144  # v  h0, chunks 8:16
OFF_V1 = 7168  # v  h1, chunks 0:16
OFF_QT0_REST = 9216  # qt h0, n 512:2048
OFF_QT1_REST = 10752  # qt h1, n 512:2048
OFF_WT = 12288  # wt h0 (1024) | wt h1 (1024)
PACK_COLS = 14336


def _kt_off(h, si):
    return (OFF_KT1 if h == 1 else OFF_KT0) + si * P


def _qt_off(h, nq):
    if nq == 0:
        return OFF_QT0_B0 if h == 0 else OFF_QT1_B0
    base = OFF_QT0_REST if h == 0 else OFF_QT1_REST
    return base + (nq - 1) * NB


def _v_off(h, c):
    if h == 1:
        return OFF_V1 + c * HD
    return (OFF_V0_LO + c * HD) if c < 8 else (OFF_V0_HI + (c - 8) * HD)


def _wt_off(h, o):
    return OFF_WT + h * DIM + o


_nc_cache = {}


def _build():
    nc = bacc.Bacc(None, target_bir_lowering=False, debug=False, num_devices=1)

    inp = nc.dram_tensor("inp", [P, PACK_COLS], F16, kind="ExternalInput").ap()
    out = nc.dram_tensor("out", [P, N // P, DIM], F16, kind="ExternalOutput").ap()

    EXPF = mybir.ActivationFunctionType.Exp
    ADD = mybir.AluOpType.add
    MULT = mybir.AluOpType.mult

    with tile.TileContext(nc) as tc:
        with (
            tc.tile_pool(name="persist", bufs=1) as persist,
            tc.tile_pool(name="e_pool", bufs=13) as e_pool,
            tc.tile_pool(name="a_pool", bufs=3) as a_pool,
            tc.tile_pool(name="small", bufs=3) as small,
            tc.tile_pool(name="y_pool", bufs=2) as y_pool,
            tc.tile_pool(name="s_ps_pool", bufs=2, space="PSUM") as s_ps_pool,
            tc.tile_pool(name="acc_ps_pool", bufs=4, space="PSUM") as acc_ps_pool,
        ):
            # One resident SBUF tile mirrors the packed dram layout 1:1 so
            # each DMA is a contiguous [128, w] slice copy and the first QK
            # group depends only on the first small DMA.
            kq_sb = persist.tile([P, PACK_COLS], F16)
            ones_dram = nc.inline_tensor(np.ones((P, P), np.float16), name="ones_const")
            ones_mat = persist.tile([P, P], F16)
            warm_sb = persist.tile([P, P], F16)

            def ld(a, b):
                nc.sync.dma_start(out=kq_sb[:, a:b], in_=inp[:, a:b])

            # PE warm-up: garbage matmuls into a PSUM tile that the first
            # real PV accumulation later overwrites with start=True.
            nc.vector.memset(warm_sb, 0.0)
            warm_ps = acc_ps_pool.tile([P, NB], F32, tag="acc")
            for w in range(WARM_MMS):
                nc.tensor.matmul(
                    warm_ps[:, 0:P], warm_sb, warm_sb, start=True, stop=True
                )

            ld(OFF_QT0_B0, OFF_KT0 + 512)  # qt h0 b0 + kt h0 chunks 0-3
            ld(OFF_KT0 + 512, OFF_V0_LO)  # kt h0 chunks 4-15
            ld(OFF_V0_LO, OFF_QT1_B0)  # v h0 lo
            nc.sync.dma_start(out=ones_mat, in_=ones_dram.ap())
            ld(OFF_QT1_B0, OFF_V0_HI)  # qt h1 b0 + kt h1 (one DMA)
            ld(OFF_V0_HI, OFF_V1)  # v h0 hi
            ld(OFF_V1, OFF_QT0_REST)  # v h1
            ld(OFF_QT0_REST, OFF_QT1_REST)  # qt h0 rest
            ld(OFF_QT1_REST, OFF_WT)  # qt h1 rest
            ld(OFF_WT, PACK_COLS)  # wt

            # X^T: normalized attention outputs, head-dim on partitions.
            xt_sb = persist.tile([P, HEADS_PER_CORE, N], F16)

            def pv_pair(o_ps, h, e_t, g):
                def fn():
                    for j in range(GC):
                        si = GC * g + j
                        nc.tensor.matmul(
                            o_ps,
                            kq_sb[:, _v_off(h, si) : _v_off(h, si) + HD],
                            e_t[:, j, :],
                            start=False,
                            stop=(si == SC - 1),
                        )

                return fn

            def fold(acc, ea, eb, eng=None):
                # acc = ea + eb elementwise fp16 (DVE runs this in 2x mode)
                with nc.allow_low_precision(reason="fp16 rowsum partials"):
                    (eng or nc.vector).tensor_add(acc, ea, eb)

            def gp_fold(a3, ea, eb):
                # independent partial on the otherwise-idle GpSimd: slow
                # (~2.1us) but consumed only by finish_vh several groups
                # later, so it never back-pressures the vector chain
                def fn():
                    fold(a3, ea, eb, nc.gpsimd)

                return fn

            def adds_pair(a2, a2g, ea, eb):
                def fn():
                    fold(a2, a2, ea)
                    fold(a2g, a2g, eb)

                return fn

            def finish_vh(o_ps, rb_ps, a2, a2g, a3, h, nsl):
                def fn():
                    a1 = small.tile([P, NB], F16, tag="a1")
                    fold(a2, a2, a2g)
                    if a3 is not None:
                        fold(a2, a2, a3)
                    fold(a1, a2[:, 0, :], a2[:, 1, :])
                    nc.tensor.matmul(rb_ps, ones_mat, a1, start=True, stop=True)
                    recip = small.tile([P, NB], F32, tag="recip")
                    nc.vector.reciprocal_approx_fast(out=recip, in_=rb_ps)
                    with nc.allow_low_precision(reason="fp16 attention output grid"):
                        nc.vector.tensor_mul(xt_sb[:, h, nsl], o_ps, recip)

                return fn

            def proj_half(nq, t, ot, y_sb, on_scalar=False):
                def fn():
                    nt = nq * (NB // P) + t
                    y_ps = acc_ps_pool.tile([P, NB], F32, tag="acc")
                    for hh in range(HEADS_PER_CORE):
                        wo = _wt_off(hh, ot * NB)
                        nc.tensor.matmul(
                            y_ps,
                            xt_sb[:, hh, nt * P : (nt + 1) * P],
                            kq_sb[:, wo : wo + NB],
                            start=(hh == 0),
                            stop=(hh == HEADS_PER_CORE - 1),
                        )
                    dst = y_sb[:, t, ot * NB : (ot + 1) * NB]
                    with nc.allow_low_precision(reason="fp16 partial projection"):
                        if on_scalar:
                            # last block only: ScalarE's exp stream is done
                            nc.scalar.activation(
                                out=dst, in_=y_ps,
                                func=mybir.ActivationFunctionType.Copy,
                            )
                        else:
                            nc.vector.tensor_copy(dst, y_ps)
                    blk = nq * (NB // P) + t
                    if on_scalar:
                        # tail: DMA each half so the first overlaps the rest
                        nc.sync.dma_start(
                            out=out[:, blk, ot * NB : (ot + 1) * NB], in_=dst
                        )
                    elif ot == 1:
                        nc.sync.dma_start(out=out[:, blk, :], in_=y_sb[:, t, :])

                return fn

            work_q = deque()
            for nq in range(NQ):
                nsl = slice(nq * NB, (nq + 1) * NB)
                for h in range(HEADS_PER_CORE):
                    last_vh = nq == NQ - 1 and h == HEADS_PER_CORE - 1
                    lag = 2 if last_vh else 4
                    qo = _qt_off(h, nq)
                    q_blk = kq_sb[:, qo : qo + NB]
                    o_ps = acc_ps_pool.tile([P, NB], F32, tag="acc")
                    a2 = a_pool.tile([P, GC, NB], F16, tag="a2")
                    a2g = a_pool.tile([P, GC, NB], F16, tag="a2g")
                    a3 = None if last_vh else a_pool.tile([P, GC, NB], F16, tag="a3")
                    rb_ps = None
                    es = []  # exp tiles in flight
                    for g in range(NG):
                        s_ps = s_ps_pool.tile([P, GC, NB], F32, tag="s")
                        for j in range(GC):
                            si = GC * g + j
                            ko = _kt_off(h, si)
                            nc.tensor.matmul(
                                s_ps[:, j, :],
                                kq_sb[:, ko : ko + P],
                                q_blk,
                                start=True,
                                stop=True,
                            )
                        e_t = e_pool.tile([P, GC, NB], F16, tag="e")
                        nc.scalar.activation(out=e_t, in_=s_ps, func=EXPF)
                        es.append(e_t)

                        # drain deferred work: one small closure per group,
                        # two when the queue runs deep
                        if work_q:
                            work_q.popleft()()
                            if len(work_q) >= 7 and work_q:
                                work_q.popleft()()

                        # PV + rowsum accumulation lag behind exp
                        pgs = [g - lag] if g >= lag else []
                        if last_vh and g == NG - 1:
                            pgs.append(NG - 2)  # tail: drain one group early
                        for pg in pgs:
                            pe = es[pg]
                            for j in range(GC):
                                si = GC * pg + j
                                nc.tensor.matmul(
                                    o_ps,
                                    kq_sb[:, _v_off(h, si) : _v_off(h, si) + HD],
                                    pe[:, j, :],
                                    start=(si == 0),
                                    stop=False,
                                )
                            if pg == 2:
                                fold(a2, es[0], pe)
                            elif pg == 3:
                                fold(a2g, es[1], pe)
                            elif pg == 4:  # lag==2 only
                                fold(a2, a2, pe)
                            elif pg == 5:  # lag==2 only
                                fold(a2g, a2g, pe)
                        if g == NG - 1:
                            rb_ps = acc_ps_pool.tile([P, NB], F32, tag="acc")
                    # defer PV of the trailing groups, remaining folds, and
                    # normalize; one independent partial runs on GpSimd
                    if last_vh:
                        work_q.append(adds_pair(a2, a2g, es[6], es[7]))
                        work_q.append(pv_pair(o_ps, h, es[7], 7))
                    else:
                        work_q.append(gp_fold(a3, es[4], es[5]))
                        work_q.append(pv_pair(o_ps, h, es[4], 4))
                        work_q.append(pv_pair(o_ps, h, es[5], 5))
                        work_q.append(adds_pair(a2, a2g, es[6], es[7]))
                        work_q.append(pv_pair(o_ps, h, es[6], 6))
                        work_q.append(pv_pair(o_ps, h, es[7], 7))
                    work_q.append(finish_vh(o_ps, rb_ps, a2, a2g, a3, h, nsl))
                y_sb = y_pool.tile([P, NB // P, DIM], F16, tag="y")
                for t in range(NB // P):
                    for ot in range(2):
                        work_q.append(
                            proj_half(nq, t, ot, y_sb, on_scalar=(nq == NQ - 1))
                        )

            while work_q:
                work_q.popleft()()

    nc.compile()
    return nc


def make_in_maps(query, key, value, proj_w):
    scale = float(HD) ** -0.5
    wt_full = np.ascontiguousarray(proj_w.T.astype(np.float32))  # [in, out]
    in_maps = []
    for core in range(N_CORES):
        b, hg = divmod(core, N_CORES // B)
        sl = slice(hg * HG, (hg + 1) * HG)
        # per-head partition-major views: [h][p][...]
        qt = (query[b].T[sl] * scale).astype(np.float16).reshape(2, P, N)
        kt = key[b].T[sl].astype(np.float16).reshape(2, P, S)
        # v[p, h, c, d] = value[c*128+p, h*128+d] -> [h][c][p][d]
        vv = (
            value[b][:, sl]
            .astype(np.float16)
            .reshape(SC, P, HEADS_PER_CORE, HD)
            .transpose(2, 0, 1, 3)
        )
        wt = wt_full[sl].astype(np.float16).reshape(2, P, DIM)

        pack = np.empty((P, PACK_COLS), dtype=np.float16)
        pack[:, OFF_QT0_B0:OFF_KT0] = qt[0, :, 0:NB]
        pack[:, OFF_KT0:OFF_V0_LO] = kt[0]
        pack[:, OFF_V0_LO:OFF_QT1_B0] = (
            vv[0, 0:8].transpose(1, 0, 2).reshape(P, 8 * HD)
        )
        pack[:, OFF_QT1_B0:OFF_KT1] = qt[1, :, 0:NB]
        pack[:, OFF_KT1:OFF_V0_HI] = kt[1]
        pack[:, OFF_V0_HI:OFF_V1] = vv[0, 8:16].transpose(1, 0, 2).reshape(P, 8 * HD)
        pack[:, OFF_V1:OFF_QT0_REST] = (
            vv[1].transpose(1, 0, 2).reshape(P, SC * HD)
        )
        pack[:, OFF_QT0_REST:OFF_QT1_REST] = qt[0, :, NB:]
        pack[:, OFF_QT1_REST:OFF_WT] = qt[1, :, NB:]
        pack[:, OFF_WT : OFF_WT + DIM] = wt[0]
        pack[:, OFF_WT + DIM :] = wt[1]
        in_maps.append({"inp": pack})
    return in_maps


def kernel(query, key, value, proj_w, proj_b):
    query = np.asarray(query)
    key = np.asarray(key)
    value = np.asarray(value)
    proj_w = np.asarray(proj_w)
    proj_b = np.asarray(proj_b)
    if "nc" not in _nc_cache:
        _nc_cache["nc"] = _build()
    nc = _nc_cache["nc"]

    in_maps = make_in_maps(query, key, value, proj_w)
    res = run_bass_kernel_spmd(nc, in_maps, list(range(N_CORES)))

    out = np.zeros((B, N, DIM), dtype=np.float32)
    for core in range(N_CORES):
        b = core // (N_CORES // B)
        # out dram is [p, chunk, o] with row n = chunk*128 + p
        part = res.results[core]["out"]
        out[b] += part.transpose(1, 0, 2).reshape(N, DIM).astype(np.float32)
    out += proj_b.astype(np.float32)
    return out


# revision 21
# speedup vs baseline: 1.2184x; 1.0083x over previous
"""Multi-head attention + output projection on 8 Trainium2 NeuronCores.

Problem (hardcoded): B=2, N=S=2048, DIM=1024, 8 heads, head_dim=128, fp32.
  out = softmax(Q K^T / sqrt(128)) V  -> reshape -> @ proj_w.T + proj_b

Sharding: data parallel on batch (2) x tensor parallel on heads (4 groups of
2 heads).  Each core computes attention for its 2 heads plus the partial
output projection restricted to its heads' columns; the host sums the 4
partial projections per batch and adds the bias.

Per-core kernel (matmul operands fp16, accumulation fp32 PSUM):
  S^T = K @ Q^T per 128-row s-chunk with s on partitions (softmax needs no
  on-chip transpose of P); exp on ScalarE (PSUM->SBUF, scale pre-applied to
  Q on host); out^T = V^T @ expS^T accumulated in PSUM.  Row sums feed an
  all-ones [128x128] matmul that colsums over partitions with the result
  broadcast to all 128 rows; reciprocal_approx_fast + multiply normalizes.

  The emission order is software-pipelined for the in-order engine queues:
  PV lags four groups behind QK/exp (hiding cross-engine semaphore
  latency), and each head's tail work (last PV pairs, rowsum matmuls,
  reciprocal, normalize) plus each block's projection are deferred into
  the NEXT head's stream so the exp pipeline on ScalarE never drains at
  head boundaries.

v2 changes over the first working version:
  - All inputs live in ONE host-packed dram tensor (partition-major,
    ordered by first use) mirrored 1:1 by one SBUF mega-tile, loaded by
    fine-grained contiguous DMAs so the first QK depends on a single small
    DMA instead of the whole input set.
  - ~20 dummy warm-up matmuls run during the input DMA window so the PE
    HAM clock gate is (mostly) released before the first real matmul.
  - Rowsum fold adds use scalar_tensor_tensor (TensorScalarPtr supports
    the DVE 4x perf mode; TensorTensor only gets 2x).
  - Projection PSUM->SBUF copies moved from VectorE to the idle GpSimd
    engine, casting to fp16; output DMAs are fp16 (host sums partials in
    fp32) and issued per 128-row chunk right after each copy.
"""

import sys
from collections import deque

sys.path.insert(0, "/opt/trn_rl_repo")

import numpy as np

import concourse.bass as bass  # noqa: F401  (engine namespaces live on nc)
import concourse.mybir as mybir
import concourse.tile as tile
from concourse import bacc
from concourse.bass_utils import run_bass_kernel_spmd

B = 2
N = 2048
S = 2048
DIM = 1024
NUM_HEADS = 8
HD = 128
N_CORES = 8
HEADS_PER_CORE = 2  # 4-way head parallel x 2-way batch parallel
HG = DIM // (NUM_HEADS // HEADS_PER_CORE)  # 256 dims per core
P = 128
SC = S // P  # 16 s-chunks
NB = 512  # query-column block
NQ = N // NB
GC = 2  # s-chunks per exp group
NG = SC // GC  # 8 groups per (head, block)
F32 = mybir.dt.float32
F16 = mybir.dt.float16

WARM_MMS = 26  # dummy matmuls to release the PE HAM clock gate during DMA-in
# (sized to keep the PE busy from engine start ~6.9us until the first input
# DMA lands ~9.8us, so the HAM 4096-cycle window flips right as the real
# matmul stream begins)

# ---- packed input layout: column offsets (fp16 elements per partition) ----
# ordered by first use inside the kernel
OFF_QT0_B0 = 0  # qt h0, n 0:512
OFF_KT0 = 512  # kt h0, s 0:2048
OFF_V0_LO = 2560  # v  h0, chunks 0:8   (8*128 d-major)
OFF_QT1_B0 = 3584  # qt h1, n 0:512
OFF_KT1 = 4096  # kt h1, s 0:2048
OFF_V0_HI = 6144  # v  h0, chunks 8:16
OFF_V1 = 7168  # v  h1, chunks 0:16
OFF_QT0_REST = 9216  # qt h0, n 512:2048
OFF_QT1_REST = 10752  # qt h1, n 512:2048
OFF_WT = 12288  # wt h0 (1024) | wt h1 (1024)
PACK_COLS = 14336


def _kt_off(h, si):
    return (OFF_KT1 if h == 1 else OFF_KT0) + si * P


def _qt_off(h, nq):
    if nq == 0:
        return OFF_QT0_B0 if h == 0 else OFF_QT1_B0
    base = OFF_QT0_REST if h == 0 else OFF_QT1_REST
    return base + (nq - 1) * NB


def _v_off(h, c):
    if h == 1:
        return OFF_V1 + c * HD
    return (OFF_V0_LO + c * HD) if c < 8 else (OFF_V0_HI + (c - 8) * HD)


def _wt_off(h, o):
    return OFF_WT + h * DIM + o


_nc_cache = {}


def _build():
    nc = bacc.Bacc(None, target_bir_lowering=False, debug=False, num_devices=1)

    inp = nc.dram_tensor("inp", [P, PACK_COLS], F16, kind="ExternalInput").ap()
    out = nc.dram_tensor("out", [P, N // P, DIM], F16, kind="ExternalOutput").ap()

    EXPF = mybir.ActivationFunctionType.Exp
    ADD = mybir.AluOpType.add
    MULT = mybir.AluOpType.mult

    with tile.TileContext(nc) as tc:
        with (
            tc.tile_pool(name="persist", bufs=1) as persist,
            tc.tile_pool(name="e_pool", bufs=13) as e_pool,
            tc.tile_pool(name="a_pool", bufs=3) as a_pool,
            tc.tile_pool(name="small", bufs=3) as small,
            tc.tile_pool(name="y_pool", bufs=2) as y_pool,
            tc.tile_pool(name="s_ps_pool", bufs=2, space="PSUM") as s_ps_pool,
            tc.tile_pool(name="acc_ps_pool", bufs=4, space="PSUM") as acc_ps_pool,
        ):
            # One resident SBUF tile mirrors the packed dram layout 1:1 so
            # each DMA is a contiguous [128, w] slice copy and the first QK
            # group depends only on the first small DMA.
            kq_sb = persist.tile([P, PACK_COLS], F16)
            ones_dram = nc.inline_tensor(np.ones((P, P), np.float16), name="ones_const")
            ones_mat = persist.tile([P, P], F16)
            warm_sb = persist.tile([P, P], F16)

            def ld(a, b):
                nc.sync.dma_start(out=kq_sb[:, a:b], in_=inp[:, a:b])

            # PE warm-up: garbage matmuls into a PSUM tile that the first
            # real PV accumulation later overwrites with start=True.
            nc.vector.memset(warm_sb, 0.0)
            warm_ps = acc_ps_pool.tile([P, NB], F32, tag="acc")
            for w in range(WARM_MMS):
                nc.tensor.matmul(
                    warm_ps[:, 0:P], warm_sb, warm_sb, start=True, stop=True
                )

            ld(OFF_QT0_B0, OFF_KT0 + 512)  # qt h0 b0 + kt h0 chunks 0-3
            ld(OFF_KT0 + 512, OFF_V0_LO)  # kt h0 chunks 4-15
            ld(OFF_V0_LO, OFF_QT1_B0)  # v h0 lo
            nc.sync.dma_start(out=ones_mat, in_=ones_dram.ap())
            ld(OFF_QT1_B0, OFF_V0_HI)  # qt h1 b0 + kt h1 (one DMA)
            ld(OFF_V0_HI, OFF_V1)  # v h0 hi
            ld(OFF_V1, OFF_QT0_REST)  # v h1
            ld(OFF_QT0_REST, OFF_QT1_REST)  # qt h0 rest
            ld(OFF_QT1_REST, OFF_WT)  # qt h1 rest
            ld(OFF_WT, PACK_COLS)  # wt

            # X^T: normalized attention outputs, head-dim on partitions.
            xt_sb = persist.tile([P, HEADS_PER_CORE, N], F16)

            def pv_pair(o_ps, h, e_t, g):
                def fn():
                    for j in range(GC):
                        si = GC * g + j
                        nc.tensor.matmul(
                            o_ps,
                            kq_sb[:, _v_off(h, si) : _v_off(h, si) + HD],
                            e_t[:, j, :],
                            start=False,
                            stop=(si == SC - 1),
                        )

                return fn

            def fold(acc, ea, eb, eng=None):
                # acc = ea + eb elementwise fp16 (DVE runs this in 2x mode)
                with nc.allow_low_precision(reason="fp16 rowsum partials"):
                    (eng or nc.vector).tensor_add(acc, ea, eb)

            def gp_fold(a3, ea, eb):
                # independent partial on the otherwise-idle GpSimd: slow
                # (~2.1us) but consumed only by finish_vh several groups
                # later, so it never back-pressures the vector chain
                def fn():
                    fold(a3, ea, eb, nc.gpsimd)

                return fn

            def adds_pair(a2, a2g, ea, eb):
                def fn():
                    fold(a2, a2, ea)
                    fold(a2g, a2g, eb)

                return fn

            def finish_vh(o_ps, rb_ps, a2, a2g, a3, h, nsl):
                def fn():
                    a1 = small.tile([P, NB], F16, tag="a1")
                    fold(a2, a2, a2g)
                    if a3 is not None:
                        fold(a2, a2, a3)
                    fold(a1, a2[:, 0, :], a2[:, 1, :])
                    nc.tensor.matmul(rb_ps, ones_mat, a1, start=True, stop=True)
                    recip = small.tile([P, NB], F32, tag="recip")
                    nc.vector.reciprocal_approx_fast(out=recip, in_=rb_ps)
                    with nc.allow_low_precision(reason="fp16 attention output grid"):
                        nc.vector.tensor_mul(xt_sb[:, h, nsl], o_ps, recip)

                return fn

            def proj_half(nq, t, ot, y_sb, tail=False):
                def fn():
                    nt = nq * (NB // P) + t
                    y_ps = acc_ps_pool.tile([P, NB], F32, tag="acc")
                    for hh in range(HEADS_PER_CORE):
                        wo = _wt_off(hh, ot * NB)
                        nc.tensor.matmul(
                            y_ps,
                            xt_sb[:, hh, nt * P : (nt + 1) * P],
                            kq_sb[:, wo : wo + NB],
                            start=(hh == 0),
                            stop=(hh == HEADS_PER_CORE - 1),
                        )
                    dst = y_sb[:, t, ot * NB : (ot + 1) * NB]
                    with nc.allow_low_precision(reason="fp16 partial projection"):
                        if tail and ot == 0:
                            # last block: exp stream is done, so ScalarE can
                            # take half the copies (parallel with VectorE)
                            nc.scalar.activation(
                                out=dst, in_=y_ps,
                                func=mybir.ActivationFunctionType.Copy,
                            )
                        else:
                            nc.vector.tensor_copy(dst, y_ps)
                    if ot == 1:
                        nc.sync.dma_start(
                            out=out[:, nq * (NB // P) + t, :], in_=y_sb[:, t, :]
                        )

                return fn

            work_q = deque()
            for nq in range(NQ):
                nsl = slice(nq * NB, (nq + 1) * NB)
                for h in range(HEADS_PER_CORE):
                    last_vh = nq == NQ - 1 and h == HEADS_PER_CORE - 1
                    lag = 2 if last_vh else 4
                    qo = _qt_off(h, nq)
                    q_blk = kq_sb[:, qo : qo + NB]
                    o_ps = acc_ps_pool.tile([P, NB], F32, tag="acc")
                    a2 = a_pool.tile([P, GC, NB], F16, tag="a2")
                    a2g = a_pool.tile([P, GC, NB], F16, tag="a2g")
                    a3 = None if last_vh else a_pool.tile([P, GC, NB], F16, tag="a3")
                    rb_ps = None
                    es = []  # exp tiles in flight
                    for g in range(NG):
                        s_ps = s_ps_pool.tile([P, GC, NB], F32, tag="s")
                        for j in range(GC):
                            si = GC * g + j
                            ko = _kt_off(h, si)
                            nc.tensor.matmul(
                                s_ps[:, j, :],
                                kq_sb[:, ko : ko + P],
                                q_blk,
                                start=True,
                                stop=True,
                            )
                        e_t = e_pool.tile([P, GC, NB], F16, tag="e")
                        nc.scalar.activation(out=e_t, in_=s_ps, func=EXPF)
                        es.append(e_t)

                        # drain deferred work: one small closure per group,
                        # two when the queue runs deep
                        if work_q:
                            work_q.popleft()()
                            if len(work_q) >= 7 and work_q:
                                work_q.popleft()()

                        # PV + rowsum accumulation lag behind exp
                        pgs = [g - lag] if g >= lag else []
                        if last_vh and g == NG - 1:
                            pgs.append(NG - 2)  # tail: drain one group early
                        for pg in pgs:
                            pe = es[pg]
                            for j in range(GC):
                                si = GC * pg + j
                                nc.tensor.matmul(
                                    o_ps,
                                    kq_sb[:, _v_off(h, si) : _v_off(h, si) + HD],
                                    pe[:, j, :],
                                    start=(si == 0),
                                    stop=False,
                                )
                            if pg == 2:
                                fold(a2, es[0], pe)
                            elif pg == 3:
                                fold(a2g, es[1], pe)
                            elif pg == 4:  # lag==2 only
                                fold(a2, a2, pe)
                            elif pg == 5:  # lag==2 only
                                fold(a2g, a2g, pe)
                        if g == NG - 1:
                            rb_ps = acc_ps_pool.tile([P, NB], F32, tag="acc")
                    # defer PV of the trailing groups, remaining folds, and
                    # normalize; one independent partial runs on GpSimd
                    if last_vh:
                        work_q.append(adds_pair(a2, a2g, es[6], es[7]))
                        work_q.append(pv_pair(o_ps, h, es[7], 7))
                    else:
                        work_q.append(gp_fold(a3, es[4], es[5]))
                        work_q.append(pv_pair(o_ps, h, es[4], 4))
                        work_q.append(pv_pair(o_ps, h, es[5], 5))
                        work_q.append(adds_pair(a2, a2g, es[6], es[7]))
                        work_q.append(pv_pair(o_ps, h, es[6], 6))
                        work_q.append(pv_pair(o_ps, h, es[7], 7))
                    work_q.append(finish_vh(o_ps, rb_ps, a2, a2g, a3, h, nsl))
                y_sb = y_pool.tile([P, NB // P, DIM], F16, tag="y")
                for t in range(NB // P):
                    for ot in range(2):
                        work_q.append(
                            proj_half(nq, t, ot, y_sb, tail=(nq == NQ - 1))
                        )

            while work_q:
                work_q.popleft()()

    nc.compile()
    return nc


def make_in_maps(query, key, value, proj_w):
    scale = float(HD) ** -0.5
    wt_full = np.ascontiguousarray(proj_w.T.astype(np.float32))  # [in, out]
    in_maps = []
    for core in range(N_CORES):
        b, hg = divmod(core, N_CORES // B)
        sl = slice(hg * HG, (hg + 1) * HG)
        # per-head partition-major views: [h][p][...]
        qt = (query[b].T[sl] * scale).astype(np.float16).reshape(2, P, N)
        kt = key[b].T[sl].astype(np.float16).reshape(2, P, S)
        # v[p, h, c, d] = value[c*128+p, h*128+d] -> [h][c][p][d]
        vv = (
            value[b][:, sl]
            .astype(np.float16)
            .reshape(SC, P, HEADS_PER_CORE, HD)
            .transpose(2, 0, 1, 3)
        )
        wt = wt_full[sl].astype(np.float16).reshape(2, P, DIM)

        pack = np.empty((P, PACK_COLS), dtype=np.float16)
        pack[:, OFF_QT0_B0:OFF_KT0] = qt[0, :, 0:NB]
        pack[:, OFF_KT0:OFF_V0_LO] = kt[0]
        pack[:, OFF_V0_LO:OFF_QT1_B0] = (
            vv[0, 0:8].transpose(1, 0, 2).reshape(P, 8 * HD)
        )
        pack[:, OFF_QT1_B0:OFF_KT1] = qt[1, :, 0:NB]
        pack[:, OFF_KT1:OFF_V0_HI] = kt[1]
        pack[:, OFF_V0_HI:OFF_V1] = vv[0, 8:16].transpose(1, 0, 2).reshape(P, 8 * HD)
        pack[:, OFF_V1:OFF_QT0_REST] = (
            vv[1].transpose(1, 0, 2).reshape(P, SC * HD)
        )
        pack[:, OFF_QT0_REST:OFF_QT1_REST] = qt[0, :, NB:]
        pack[:, OFF_QT1_REST:OFF_WT] = qt[1, :, NB:]
        pack[:, OFF_WT : OFF_WT + DIM] = wt[0]
        pack[:, OFF_WT + DIM :] = wt[1]
        in_maps.append({"inp": pack})
    return in_maps


def kernel(query, key, value, proj_w, proj_b):
    query = np.asarray(query)
    key = np.asarray(key)
    value = np.asarray(value)
    proj_w = np.asarray(proj_w)
    proj_b = np.asarray(proj_b)
    if "nc" not in _nc_cache:
        _nc_cache["nc"] = _build()
    nc = _nc_cache["nc"]

    in_maps = make_in_maps(query, key, value, proj_w)
    res = run_bass_kernel_spmd(nc, in_maps, list(range(N_CORES)))

    out = np.zeros((B, N, DIM), dtype=np.float32)
    for core in range(N_CORES):
        b = core // (N_CORES // B)
        # out dram is [p, chunk, o] with row n = chunk*128 + p
        part = res.results[core]["out"]
        out[b] += part.transpose(1, 0, 2).reshape(N, DIM).astype(np.float32)
    out += proj_b.astype(np.float32)
    return out


# revision 25
# speedup vs baseline: 1.2374x; 1.0156x over previous
"""Multi-head attention + output projection on 8 Trainium2 NeuronCores.

Problem (hardcoded): B=2, N=S=2048, DIM=1024, 8 heads, head_dim=128, fp32.
  out = softmax(Q K^T / sqrt(128)) V  -> reshape -> @ proj_w.T + proj_b

Sharding: data parallel on batch (2) x tensor parallel on heads (4 groups of
2 heads).  Each core computes attention for its 2 heads plus the partial
output projection restricted to its heads' columns; the host sums the 4
partial projections per batch and adds the bias.

Per-core kernel (matmul operands fp16, accumulation fp32 PSUM):
  S^T = K @ Q^T per 128-row s-chunk with s on partitions (softmax needs no
  on-chip transpose of P); exp on ScalarE (PSUM->SBUF, scale pre-applied to
  Q on host); out^T = V^T @ expS^T accumulated in PSUM.  Row sums feed an
  all-ones [128x128] matmul that colsums over partitions with the result
  broadcast to all 128 rows; reciprocal_approx_fast + multiply normalizes.

  The emission order is software-pipelined for the in-order engine queues:
  PV lags four groups behind QK/exp (hiding cross-engine semaphore
  latency), and each head's tail work (last PV pairs, rowsum matmuls,
  reciprocal, normalize) plus each block's projection are deferred into
  the NEXT head's stream so the exp pipeline on ScalarE never drains at
  head boundaries.

v2 changes over the first working version:
  - All inputs live in ONE host-packed dram tensor (partition-major,
    ordered by first use) mirrored 1:1 by one SBUF mega-tile, loaded by
    fine-grained contiguous DMAs so the first QK depends on a single small
    DMA instead of the whole input set.
  - ~20 dummy warm-up matmuls run during the input DMA window so the PE
    HAM clock gate is (mostly) released before the first real matmul.
  - Rowsum fold adds use scalar_tensor_tensor (TensorScalarPtr supports
    the DVE 4x perf mode; TensorTensor only gets 2x).
  - Projection PSUM->SBUF copies moved from VectorE to the idle GpSimd
    engine, casting to fp16; output DMAs are fp16 (host sums partials in
    fp32) and issued per 128-row chunk right after each copy.
"""

import sys
from collections import deque

sys.path.insert(0, "/opt/trn_rl_repo")

import numpy as np

import concourse.bass as bass  # noqa: F401  (engine namespaces live on nc)
import concourse.mybir as mybir
import concourse.tile as tile
from concourse import bacc
from concourse.bass_utils import run_bass_kernel_spmd

B = 2
N = 2048
S = 2048
DIM = 1024
NUM_HEADS = 8
HD = 128
N_CORES = 8
HEADS_PER_CORE = 2  # 4-way head parallel x 2-way batch parallel
HG = DIM // (NUM_HEADS // HEADS_PER_CORE)  # 256 dims per core
P = 128
SC = S // P  # 16 s-chunks
NB = 512  # query-column block
NQ = N // NB
GC = 2  # s-chunks per exp group
NG = SC // GC  # 8 groups per (head, block)
F32 = mybir.dt.float32
F16 = mybir.dt.float16

WARM_MMS = 26  # dummy matmuls to release the PE HAM clock gate during DMA-in
# (sized to keep the PE busy from engine start ~6.9us until the first input
# DMA lands ~9.8us, so the HAM 4096-cycle window flips right as the real
# matmul stream begins)

# ---- packed input layout: column offsets (fp16 elements per partition) ----
# ordered by first use inside the kernel
OFF_QT0_B0 = 0  # qt h0, n 0:512
OFF_KT0 = 512  # kt h0, s 0:2048
OFF_V0_LO = 2560  # v  h0, chunks 0:8   (8*128 d-major)
OFF_QT1_B0 = 3584  # qt h1, n 0:512
OFF_KT1 = 4096  # kt h1, s 0:2048
OFF_V0_HI = 6144  # v  h0, chunks 8:16
OFF_V1 = 7168  # v  h1, chunks 0:16
OFF_QT0_REST = 9216  # qt h0, n 512:2048
OFF_QT1_REST = 10752  # qt h1, n 512:2048
OFF_WT = 12288  # wt h0 (1024) | wt h1 (1024)
PACK_COLS = 14336


def _kt_off(h, si):
    return (OFF_KT1 if h == 1 else OFF_KT0) + si * P


def _qt_off(h, nq):
    if nq == 0:
        return OFF_QT0_B0 if h == 0 else OFF_QT1_B0
    base = OFF_QT0_REST if h == 0 else OFF_QT1_REST
    return base + (nq - 1) * NB


def _v_off(h, c):
    if h == 1:
        return OFF_V1 + c * HD
    return (OFF_V0_LO + c * HD) if c < 8 else (OFF_V0_HI + (c - 8) * HD)


def _wt_off(h, o):
    return OFF_WT + h * DIM + o


_nc_cache = {}


def _build():
    nc = bacc.Bacc(None, target_bir_lowering=False, debug=False, num_devices=1)

    inp = nc.dram_tensor("inp", [P, PACK_COLS], F16, kind="ExternalInput").ap()
    out = nc.dram_tensor("out", [P, N // P, DIM], F16, kind="ExternalOutput").ap()

    EXPF = mybir.ActivationFunctionType.Exp
    ADD = mybir.AluOpType.add
    MULT = mybir.AluOpType.mult

    with tile.TileContext(nc) as tc:
        with (
            tc.tile_pool(name="persist", bufs=1) as persist,
            tc.tile_pool(name="e_pool", bufs=13) as e_pool,
            tc.tile_pool(name="a_pool", bufs=3) as a_pool,
            tc.tile_pool(name="small", bufs=3) as small,
            tc.tile_pool(name="y_pool", bufs=2) as y_pool,
            tc.tile_pool(name="s_ps_pool", bufs=2, space="PSUM") as s_ps_pool,
            tc.tile_pool(name="acc_ps_pool", bufs=4, space="PSUM") as acc_ps_pool,
        ):
            # One resident SBUF tile mirrors the packed dram layout 1:1 so
            # each DMA is a contiguous [128, w] slice copy and the first QK
            # group depends only on the first small DMA.
            kq_sb = persist.tile([P, PACK_COLS], F16)
            ones_dram = nc.inline_tensor(np.ones((P, P), np.float16), name="ones_const")
            ones_mat = persist.tile([P, P], F16)
            warm_sb = persist.tile([P, P], F16)

            def ld(a, b):
                nc.sync.dma_start(out=kq_sb[:, a:b], in_=inp[:, a:b])

            # PE warm-up: garbage matmuls into a PSUM tile that the first
            # real PV accumulation later overwrites with start=True.
            nc.vector.memset(warm_sb, 0.0)
            warm_ps = acc_ps_pool.tile([P, NB], F32, tag="acc")
            for w in range(WARM_MMS):
                nc.tensor.matmul(
                    warm_ps[:, 0:P], warm_sb, warm_sb, start=True, stop=True
                )

            ld(OFF_QT0_B0, OFF_KT0 + 512)  # qt h0 b0 + kt h0 chunks 0-3
            ld(OFF_KT0 + 512, OFF_V0_LO)  # kt h0 chunks 4-15
            ld(OFF_V0_LO, OFF_QT1_B0)  # v h0 lo
            nc.sync.dma_start(out=ones_mat, in_=ones_dram.ap())
            ld(OFF_QT1_B0, OFF_V0_HI)  # qt h1 b0 + kt h1 (one DMA)
            ld(OFF_V0_HI, OFF_V1)  # v h0 hi
            ld(OFF_V1, OFF_QT0_REST)  # v h1
            ld(OFF_QT0_REST, OFF_QT1_REST)  # qt h0 rest
            ld(OFF_QT1_REST, OFF_WT)  # qt h1 rest
            ld(OFF_WT, PACK_COLS)  # wt

            # X^T: normalized attention outputs, head-dim on partitions.
            xt_sb = persist.tile([P, HEADS_PER_CORE, N], F16)

            def pv_pair(o_ps, h, e_t, g):
                def fn():
                    for j in range(GC):
                        si = GC * g + j
                        nc.tensor.matmul(
                            o_ps,
                            kq_sb[:, _v_off(h, si) : _v_off(h, si) + HD],
                            e_t[:, j, :],
                            start=False,
                            stop=(si == SC - 1),
                        )

                return fn

            def fold(acc, ea, eb, eng=None):
                # acc = ea + eb elementwise fp16 (DVE runs this in 2x mode)
                with nc.allow_low_precision(reason="fp16 rowsum partials"):
                    (eng or nc.vector).tensor_add(acc, ea, eb)

            def gp_fold(a3, ea, eb):
                # independent partial on the otherwise-idle GpSimd: slow
                # (~2.1us) but consumed only by finish_vh several groups
                # later, so it never back-pressures the vector chain
                def fn():
                    fold(a3, ea, eb, nc.gpsimd)

                return fn

            def adds_pair(a2, a2g, ea, eb):
                def fn():
                    fold(a2, a2, ea)
                    fold(a2g, a2g, eb)

                return fn

            def finish_vh(o_ps, rb_ps, a2, a2g, a3, h, nsl):
                def fn():
                    a1 = small.tile([P, NB], F16, tag="a1")
                    fold(a2, a2, a2g)
                    if a3 is not None:
                        fold(a2, a2, a3)
                    fold(a1, a2[:, 0, :], a2[:, 1, :])
                    nc.tensor.matmul(rb_ps, ones_mat, a1, start=True, stop=True)
                    recip = small.tile([P, NB], F32, tag="recip")
                    nc.vector.reciprocal_approx_fast(out=recip, in_=rb_ps)
                    with nc.allow_low_precision(reason="fp16 attention output grid"):
                        nc.vector.tensor_mul(xt_sb[:, h, nsl], o_ps, recip)

                return fn

            def finish_last(o_ps, rb_ps, a2, a2g, h, nsl):
                # tail variant, minimizing serial latency after the last exp:
                # a1-fold replaced by two accumulating ones-matmuls, normalize
                # split in halves so the first projection rows unblock early
                def fn():
                    fold(a2, a2, a2g)
                    nc.tensor.matmul(
                        rb_ps, ones_mat, a2[:, 0, :], start=True, stop=False
                    )
                    nc.tensor.matmul(
                        rb_ps, ones_mat, a2[:, 1, :], start=False, stop=True
                    )
                    recip = small.tile([P, NB], F32, tag="recip")
                    nc.vector.reciprocal_approx_fast(out=recip, in_=rb_ps)
                    hb = NB // 2
                    with nc.allow_low_precision(reason="fp16 attention output grid"):
                        nc.vector.tensor_mul(
                            xt_sb[:, h, nsl.start : nsl.start + hb],
                            o_ps[:, 0:hb],
                            recip[:, 0:hb],
                        )
                        nc.vector.tensor_mul(
                            xt_sb[:, h, nsl.start + hb : nsl.stop],
                            o_ps[:, hb:],
                            recip[:, hb:],
                        )

                return fn

            def proj_half(nq, t, ot, y_sb, tail=False):
                def fn():
                    nt = nq * (NB // P) + t
                    y_ps = acc_ps_pool.tile([P, NB], F32, tag="acc")
                    for hh in range(HEADS_PER_CORE):
                        wo = _wt_off(hh, ot * NB)
                        nc.tensor.matmul(
                            y_ps,
                            xt_sb[:, hh, nt * P : (nt + 1) * P],
                            kq_sb[:, wo : wo + NB],
                            start=(hh == 0),
                            stop=(hh == HEADS_PER_CORE - 1),
                        )
                    dst = y_sb[:, t, ot * NB : (ot + 1) * NB]
                    with nc.allow_low_precision(reason="fp16 partial projection"):
                        if tail and ot == 0:
                            # last block: exp stream is done, so ScalarE can
                            # take half the copies (parallel with VectorE)
                            nc.scalar.activation(
                                out=dst, in_=y_ps,
                                func=mybir.ActivationFunctionType.Copy,
                            )
                        else:
                            nc.vector.tensor_copy(dst, y_ps)
                    if ot == 1:
                        nc.sync.dma_start(
                            out=out[:, nq * (NB // P) + t, :], in_=y_sb[:, t, :]
                        )

                return fn

            work_q = deque()
            for nq in range(NQ):
                nsl = slice(nq * NB, (nq + 1) * NB)
                for h in range(HEADS_PER_CORE):
                    last_vh = nq == NQ - 1 and h == HEADS_PER_CORE - 1
                    lag = 2 if last_vh else 4
                    qo = _qt_off(h, nq)
                    q_blk = kq_sb[:, qo : qo + NB]
                    o_ps = acc_ps_pool.tile([P, NB], F32, tag="acc")
                    a2 = a_pool.tile([P, GC, NB], F16, tag="a2")
                    a2g = a_pool.tile([P, GC, NB], F16, tag="a2g")
                    a3 = None if last_vh else a_pool.tile([P, GC, NB], F16, tag="a3")
                    rb_ps = None
                    es = []  # exp tiles in flight
                    for g in range(NG):
                        s_ps = s_ps_pool.tile([P, GC, NB], F32, tag="s")
                        for j in range(GC):
                            si = GC * g + j
                            ko = _kt_off(h, si)
                            nc.tensor.matmul(
                                s_ps[:, j, :],
                                kq_sb[:, ko : ko + P],
                                q_blk,
                                start=True,
                                stop=True,
                            )
                        e_t = e_pool.tile([P, GC, NB], F16, tag="e")
                        nc.scalar.activation(out=e_t, in_=s_ps, func=EXPF)
                        es.append(e_t)

                        # drain deferred work: one small closure per group,
                        # two when the queue runs deep or during the tail
                        if work_q:
                            work_q.popleft()()
                            if (len(work_q) >= 7 or last_vh) and work_q:
                                work_q.popleft()()

                        # PV + rowsum accumulation lag behind exp
                        pgs = [g - lag] if g >= lag else []
                        if last_vh and g == NG - 1:
                            pgs.append(NG - 2)  # tail: drain one group early
                        for pg in pgs:
                            pe = es[pg]
                            for j in range(GC):
                                si = GC * pg + j
                                nc.tensor.matmul(
                                    o_ps,
                                    kq_sb[:, _v_off(h, si) : _v_off(h, si) + HD],
                                    pe[:, j, :],
                                    start=(si == 0),
                                    stop=False,
                                )
                            if pg == 2:
                                fold(a2, es[0], pe)
                            elif pg == 3:
                                fold(a2g, es[1], pe)
                            elif pg == 4:  # lag==2 only
                                fold(a2, a2, pe)
                            elif pg == 5:  # lag==2 only
                                fold(a2g, a2g, pe)
                        if last_vh and g == NG - 1:
                            # e6's exp is already done: fold it inline so only
                            # e7's fold remains on the post-exp critical path
                            fold(a2, a2, es[6])
                        if g == NG - 1:
                            rb_ps = acc_ps_pool.tile([P, NB], F32, tag="acc")
                    # defer PV of the trailing groups, remaining folds, and
                    # normalize; one independent partial runs on GpSimd
                    if last_vh:
                        def tail_fold(a2g=a2g, e7=es[7]):
                            fold(a2g, a2g, e7)

                        work_q.append(tail_fold)
                        work_q.append(pv_pair(o_ps, h, es[7], 7))
                        work_q.append(finish_last(o_ps, rb_ps, a2, a2g, h, nsl))
                    else:
                        work_q.append(gp_fold(a3, es[4], es[5]))
                        work_q.append(pv_pair(o_ps, h, es[4], 4))
                        work_q.append(pv_pair(o_ps, h, es[5], 5))
                        work_q.append(adds_pair(a2, a2g, es[6], es[7]))
                        work_q.append(pv_pair(o_ps, h, es[6], 6))
                        work_q.append(pv_pair(o_ps, h, es[7], 7))
                        work_q.append(finish_vh(o_ps, rb_ps, a2, a2g, a3, h, nsl))
                y_sb = y_pool.tile([P, NB // P, DIM], F16, tag="y")
                for t in range(NB // P):
                    for ot in range(2):
                        work_q.append(
                            proj_half(nq, t, ot, y_sb, tail=(nq == NQ - 1))
                        )

            while work_q:
                work_q.popleft()()

    nc.compile()
    return nc


def make_in_maps(query, key, value, proj_w):
    scale = float(HD) ** -0.5
    wt_full = np.ascontiguousarray(proj_w.T.astype(np.float32))  # [in, out]
    in_maps = []
    for core in range(N_CORES):
        b, hg = divmod(core, N_CORES // B)
        sl = slice(hg * HG, (hg + 1) * HG)
        # per-head partition-major views: [h][p][...]
        qt = (query[b].T[sl] * scale).astype(np.float16).reshape(2, P, N)
        kt = key[b].T[sl].astype(np.float16).reshape(2, P, S)
        # v[p, h, c, d] = value[c*128+p, h*128+d] -> [h][c][p][d]
        vv = (
            value[b][:, sl]
            .astype(np.float16)
            .reshape(SC, P, HEADS_PER_CORE, HD)
            .transpose(2, 0, 1, 3)
        )
        wt = wt_full[sl].astype(np.float16).reshape(2, P, DIM)

        pack = np.empty((P, PACK_COLS), dtype=np.float16)
        pack[:, OFF_QT0_B0:OFF_KT0] = qt[0, :, 0:NB]
        pack[:, OFF_KT0:OFF_V0_LO] = kt[0]
        pack[:, OFF_V0_LO:OFF_QT1_B0] = (
            vv[0, 0:8].transpose(1, 0, 2).reshape(P, 8 * HD)
        )
        pack[:, OFF_QT1_B0:OFF_KT1] = qt[1, :, 0:NB]
        pack[:, OFF_KT1:OFF_V0_HI] = kt[1]
        pack[:, OFF_V0_HI:OFF_V1] = vv[0, 8:16].transpose(1, 0, 2).reshape(P, 8 * HD)
        pack[:, OFF_V1:OFF_QT0_REST] = (
            vv[1].transpose(1, 0, 2).reshape(P, SC * HD)
        )
        pack[:, OFF_QT0_REST:OFF_QT1_REST] = qt[0, :, NB:]
        pack[:, OFF_QT1_REST:OFF_WT] = qt[1, :, NB:]
        pack[:, OFF_WT : OFF_WT + DIM] = wt[0]
        pack[:, OFF_WT + DIM :] = wt[1]
        in_maps.append({"inp": pack})
    return in_maps


def kernel(query, key, value, proj_w, proj_b):
    query = np.asarray(query)
    key = np.asarray(key)
    value = np.asarray(value)
    proj_w = np.asarray(proj_w)
    proj_b = np.asarray(proj_b)
    if "nc" not in _nc_cache:
        _nc_cache["nc"] = _build()
    nc = _nc_cache["nc"]

    in_maps = make_in_maps(query, key, value, proj_w)
    res = run_bass_kernel_spmd(nc, in_maps, list(range(N_CORES)))

    out = np.zeros((B, N, DIM), dtype=np.float32)
    for core in range(N_CORES):
        b = core // (N_CORES // B)
        # out dram is [p, chunk, o] with row n = chunk*128 + p
        part = res.results[core]["out"]
        out[b] += part.transpose(1, 0, 2).reshape(N, DIM).astype(np.float32)
    out += proj_b.astype(np.float32)
    return out


# revision 30
# speedup vs baseline: 1.2533x; 1.0128x over previous
"""Multi-head attention + output projection on 8 Trainium2 NeuronCores.

Problem (hardcoded): B=2, N=S=2048, DIM=1024, 8 heads, head_dim=128, fp32.
  out = softmax(Q K^T / sqrt(128)) V  -> reshape -> @ proj_w.T + proj_b

Sharding: data parallel on batch (2) x tensor parallel on heads (4 groups of
2 heads).  Each core computes attention for its 2 heads plus the partial
output projection restricted to its heads' columns; the host sums the 4
partial projections per batch and adds the bias.

Per-core kernel (matmul operands fp16, accumulation fp32 PSUM):
  S^T = K @ Q^T per 128-row s-chunk with s on partitions (softmax needs no
  on-chip transpose of P); exp on ScalarE (PSUM->SBUF, scale pre-applied to
  Q on host); out^T = V^T @ expS^T accumulated in PSUM.  Row sums feed an
  all-ones [128x128] matmul that colsums over partitions with the result
  broadcast to all 128 rows; reciprocal_approx_fast + multiply normalizes.

  The emission order is software-pipelined for the in-order engine queues:
  PV lags four groups behind QK/exp (hiding cross-engine semaphore
  latency), and each head's tail work (last PV pairs, rowsum matmuls,
  reciprocal, normalize) plus each block's projection are deferred into
  the NEXT head's stream so the exp pipeline on ScalarE never drains at
  head boundaries.

v2 changes over the first working version:
  - All inputs live in ONE host-packed dram tensor (partition-major,
    ordered by first use) mirrored 1:1 by one SBUF mega-tile, loaded by
    fine-grained contiguous DMAs so the first QK depends on a single small
    DMA instead of the whole input set.
  - ~20 dummy warm-up matmuls run during the input DMA window so the PE
    HAM clock gate is (mostly) released before the first real matmul.
  - Rowsum fold adds use scalar_tensor_tensor (TensorScalarPtr supports
    the DVE 4x perf mode; TensorTensor only gets 2x).
  - Projection PSUM->SBUF copies moved from VectorE to the idle GpSimd
    engine, casting to fp16; output DMAs are fp16 (host sums partials in
    fp32) and issued per 128-row chunk right after each copy.
"""

import sys
from collections import deque

sys.path.insert(0, "/opt/trn_rl_repo")

import numpy as np

import concourse.bass as bass  # noqa: F401  (engine namespaces live on nc)
import concourse.mybir as mybir
import concourse.tile as tile
from concourse import bacc
from concourse.bass_utils import run_bass_kernel_spmd

B = 2
N = 2048
S = 2048
DIM = 1024
NUM_HEADS = 8
HD = 128
N_CORES = 8
HEADS_PER_CORE = 2  # 4-way head parallel x 2-way batch parallel
HG = DIM // (NUM_HEADS // HEADS_PER_CORE)  # 256 dims per core
P = 128
SC = S // P  # 16 s-chunks
NB = 512  # query-column block
NQ = N // NB
GC = 2  # s-chunks per exp group
NG = SC // GC  # 8 groups per (head, block)
F32 = mybir.dt.float32
F16 = mybir.dt.float16

WARM_MMS = 26  # dummy matmuls to release the PE HAM clock gate during DMA-in
# (sized to keep the PE busy from engine start ~6.9us until the first input
# DMA lands ~9.8us, so the HAM 4096-cycle window flips right as the real
# matmul stream begins)

# ---- packed input layout: column offsets (fp16 elements per partition) ----
# ordered by first use inside the kernel
OFF_QT0_B0 = 0  # qt h0, n 0:512
OFF_KT0 = 512  # kt h0, s 0:2048
OFF_V0_LO = 2560  # v  h0, chunks 0:8   (8*128 d-major)
OFF_QT1_B0 = 3584  # qt h1, n 0:512
OFF_KT1 = 4096  # kt h1, s 0:2048
OFF_V0_HI = 6144  # v  h0, chunks 8:16
OFF_V1 = 7168  # v  h1, chunks 0:16
OFF_QT0_REST = 9216  # qt h0, n 512:2048
OFF_QT1_REST = 10752  # qt h1, n 512:2048
OFF_WT = 12288  # wt h0 (1024) | wt h1 (1024)
PACK_COLS = 14336


def _kt_off(h, si):
    return (OFF_KT1 if h == 1 else OFF_KT0) + si * P


def _qt_off(h, nq):
    if nq == 0:
        return OFF_QT0_B0 if h == 0 else OFF_QT1_B0
    base = OFF_QT0_REST if h == 0 else OFF_QT1_REST
    return base + (nq - 1) * NB


def _v_off(h, c):
    if h == 1:
        return OFF_V1 + c * HD
    return (OFF_V0_LO + c * HD) if c < 8 else (OFF_V0_HI + (c - 8) * HD)


def _wt_off(h, o):
    return OFF_WT + h * DIM + o


_nc_cache = {}


def _build():
    nc = bacc.Bacc(None, target_bir_lowering=False, debug=False, num_devices=1)

    inp = nc.dram_tensor("inp", [P, PACK_COLS], F16, kind="ExternalInput").ap()
    out = nc.dram_tensor("out", [P, N // P, DIM], F16, kind="ExternalOutput").ap()

    EXPF = mybir.ActivationFunctionType.Exp
    ADD = mybir.AluOpType.add
    MULT = mybir.AluOpType.mult

    with tile.TileContext(nc) as tc:
        with (
            tc.tile_pool(name="persist", bufs=1) as persist,
            tc.tile_pool(name="e_pool", bufs=13) as e_pool,
            tc.tile_pool(name="a_pool", bufs=3) as a_pool,
            tc.tile_pool(name="small", bufs=3) as small,
            tc.tile_pool(name="y_pool", bufs=3) as y_pool,
            tc.tile_pool(name="s_ps_pool", bufs=2, space="PSUM") as s_ps_pool,
            tc.tile_pool(name="acc_ps_pool", bufs=4, space="PSUM") as acc_ps_pool,
        ):
            # One resident SBUF tile mirrors the packed dram layout 1:1 so
            # each DMA is a contiguous [128, w] slice copy and the first QK
            # group depends only on the first small DMA.
            kq_sb = persist.tile([P, PACK_COLS], F16)
            ones_dram = nc.inline_tensor(np.ones((P, P), np.float16), name="ones_const")
            ones_mat = persist.tile([P, P], F16)
            warm_sb = persist.tile([P, P], F16)

            def ld(a, b):
                nc.sync.dma_start(out=kq_sb[:, a:b], in_=inp[:, a:b])

            # PE warm-up: garbage matmuls into a PSUM tile that the first
            # real PV accumulation later overwrites with start=True.
            nc.vector.memset(warm_sb, 0.0)
            warm_ps = acc_ps_pool.tile([P, NB], F32, tag="acc")
            for w in range(WARM_MMS):
                nc.tensor.matmul(
                    warm_ps[:, 0:P], warm_sb, warm_sb, start=True, stop=True
                )

            ld(OFF_QT0_B0, OFF_KT0 + 512)  # qt h0 b0 + kt h0 chunks 0-3
            ld(OFF_KT0 + 512, OFF_V0_LO)  # kt h0 chunks 4-15
            ld(OFF_V0_LO, OFF_QT1_B0)  # v h0 lo
            nc.sync.dma_start(out=ones_mat, in_=ones_dram.ap())
            ld(OFF_QT1_B0, OFF_V0_HI)  # qt h1 b0 + kt h1 (one DMA)
            ld(OFF_V0_HI, OFF_V1)  # v h0 hi
            ld(OFF_V1, OFF_QT0_REST)  # v h1
            ld(OFF_QT0_REST, OFF_QT1_REST)  # qt h0 rest
            ld(OFF_QT1_REST, OFF_WT)  # qt h1 rest
            ld(OFF_WT, PACK_COLS)  # wt

            # X^T: normalized attention outputs, head-dim on partitions.
            xt_sb = persist.tile([P, HEADS_PER_CORE, N], F16)

            def pv_pair(o_ps, h, e_t, g):
                def fn():
                    for j in range(GC):
                        si = GC * g + j
                        nc.tensor.matmul(
                            o_ps,
                            kq_sb[:, _v_off(h, si) : _v_off(h, si) + HD],
                            e_t[:, j, :],
                            start=False,
                            stop=(si == SC - 1),
                        )

                return fn

            def fold(acc, ea, eb, eng=None):
                # acc = ea + eb elementwise fp16 (DVE runs this in 2x mode)
                with nc.allow_low_precision(reason="fp16 rowsum partials"):
                    (eng or nc.vector).tensor_add(acc, ea, eb)

            def gp_fold(a3, ea, eb):
                # independent partial on the otherwise-idle GpSimd: slow
                # (~2.1us) but consumed only by finish_vh several groups
                # later, so it never back-pressures the vector chain
                def fn():
                    fold(a3, ea, eb, nc.gpsimd)

                return fn

            def adds_pair(a2, a2g, ea, eb):
                def fn():
                    fold(a2, a2, ea)
                    fold(a2g, a2g, eb)

                return fn

            def finish_vh(o_ps, rb_ps, a2, a2g, a3, h, nsl):
                def fn():
                    a1 = small.tile([P, NB], F16, tag="a1")
                    fold(a2, a2, a2g)
                    if a3 is not None:
                        fold(a2, a2, a3)
                    fold(a1, a2[:, 0, :], a2[:, 1, :])
                    nc.tensor.matmul(rb_ps, ones_mat, a1, start=True, stop=True)
                    recip = small.tile([P, NB], F32, tag="recip")
                    nc.vector.reciprocal_approx_fast(out=recip, in_=rb_ps)
                    with nc.allow_low_precision(reason="fp16 attention output grid"):
                        nc.vector.tensor_mul(xt_sb[:, h, nsl], o_ps, recip)

                return fn

            def finish_last(o_ps, rb_ps, a2, a2g, h, nsl):
                # tail variant, minimizing serial latency after the last exp:
                # a1-fold replaced by two accumulating ones-matmuls, normalize
                # split in halves so the first projection rows unblock early
                def fn():
                    fold(a2, a2, a2g)
                    nc.tensor.matmul(
                        rb_ps, ones_mat, a2[:, 0, :], start=True, stop=False
                    )
                    nc.tensor.matmul(
                        rb_ps, ones_mat, a2[:, 1, :], start=False, stop=True
                    )
                    recip = small.tile([P, NB], F32, tag="recip")
                    nc.vector.reciprocal_approx_fast(out=recip, in_=rb_ps)
                    hb = NB // 2
                    with nc.allow_low_precision(reason="fp16 attention output grid"):
                        nc.vector.tensor_mul(
                            xt_sb[:, h, nsl.start : nsl.start + hb],
                            o_ps[:, 0:hb],
                            recip[:, 0:hb],
                        )
                        nc.vector.tensor_mul(
                            xt_sb[:, h, nsl.start + hb : nsl.stop],
                            o_ps[:, hb:],
                            recip[:, hb:],
                        )

                return fn

            def proj_half(nq, t, ot, y_sb, tail=False):
                def fn():
                    nt = nq * (NB // P) + t
                    y_ps = acc_ps_pool.tile([P, NB], F32, tag="acc")
                    for hh in range(HEADS_PER_CORE):
                        wo = _wt_off(hh, ot * NB)
                        nc.tensor.matmul(
                            y_ps,
                            xt_sb[:, hh, nt * P : (nt + 1) * P],
                            kq_sb[:, wo : wo + NB],
                            start=(hh == 0),
                            stop=(hh == HEADS_PER_CORE - 1),
                        )
                    dst = y_sb[:, t, ot * NB : (ot + 1) * NB]
                    with nc.allow_low_precision(reason="fp16 partial projection"):
                        if tail and ot == 0:
                            # last block: exp stream is done, so ScalarE can
                            # take half the copies (parallel with VectorE)
                            nc.scalar.activation(
                                out=dst, in_=y_ps,
                                func=mybir.ActivationFunctionType.Copy,
                            )
                        else:
                            nc.vector.tensor_copy(dst, y_ps)
                    if ot == 1:
                        nc.sync.dma_start(
                            out=out[:, nq * (NB // P) + t, :], in_=y_sb[:, t, :]
                        )

                return fn

            work_q = deque()
            for nq in range(NQ):
                nsl = slice(nq * NB, (nq + 1) * NB)
                for h in range(HEADS_PER_CORE):
                    last_vh = nq == NQ - 1 and h == HEADS_PER_CORE - 1
                    # tight pipeline for the first head (the deferred-work
                    # queue is still empty, so the PE starves at lag 4) and
                    # for the last head (shortens the tail chain)
                    tight = last_vh or (nq == 0 and h == 0)
                    lag = 2 if tight else 4
                    qo = _qt_off(h, nq)
                    q_blk = kq_sb[:, qo : qo + NB]
                    o_ps = acc_ps_pool.tile([P, NB], F32, tag="acc")
                    a2 = a_pool.tile([P, GC, NB], F16, tag="a2")
                    a2g = a_pool.tile([P, GC, NB], F16, tag="a2g")
                    a3 = None if tight else a_pool.tile([P, GC, NB], F16, tag="a3")
                    rb_ps = None
                    es = []  # exp tiles in flight
                    for g in range(NG):
                        s_ps = s_ps_pool.tile([P, GC, NB], F32, tag="s")
                        for j in range(GC):
                            si = GC * g + j
                            ko = _kt_off(h, si)
                            nc.tensor.matmul(
                                s_ps[:, j, :],
                                kq_sb[:, ko : ko + P],
                                q_blk,
                                start=True,
                                stop=True,
                            )
                        e_t = e_pool.tile([P, GC, NB], F16, tag="e")
                        nc.scalar.activation(out=e_t, in_=s_ps, func=EXPF)
                        es.append(e_t)

                        # drain deferred work: one small closure per group,
                        # two when the queue runs deep or during the tail
                        if work_q:
                            work_q.popleft()()
                            if (len(work_q) >= 7 or last_vh) and work_q:
                                work_q.popleft()()

                        # PV + rowsum accumulation lag behind exp
                        pgs = [g - lag] if g >= lag else []
                        if tight and g == NG - 1:
                            pgs.append(NG - 2)  # drain one group early
                        for pg in pgs:
                            pe = es[pg]
                            for j in range(GC):
                                si = GC * pg + j
                                nc.tensor.matmul(
                                    o_ps,
                                    kq_sb[:, _v_off(h, si) : _v_off(h, si) + HD],
                                    pe[:, j, :],
                                    start=(si == 0),
                                    stop=False,
                                )
                            if pg == 2:
                                fold(a2, es[0], pe)
                            elif pg == 3:
                                fold(a2g, es[1], pe)
                            elif pg == 4:  # lag==2 only
                                fold(a2, a2, pe)
                            elif pg == 5:  # lag==2 only
                                fold(a2g, a2g, pe)
                        if tight and g == NG - 1:
                            # e6's exp is already done: fold it inline so only
                            # e7's fold remains on the post-exp critical path
                            fold(a2, a2, es[6])
                        if g == NG - 1:
                            rb_ps = acc_ps_pool.tile([P, NB], F32, tag="acc")
                    # defer PV of the trailing groups, remaining folds, and
                    # normalize; one independent partial runs on GpSimd
                    if tight:
                        def tail_fold(a2g=a2g, e7=es[7]):
                            fold(a2g, a2g, e7)

                        work_q.append(tail_fold)
                        work_q.append(pv_pair(o_ps, h, es[7], 7))
                        work_q.append(finish_last(o_ps, rb_ps, a2, a2g, h, nsl))
                    else:
                        work_q.append(gp_fold(a3, es[4], es[5]))
                        work_q.append(pv_pair(o_ps, h, es[4], 4))
                        work_q.append(pv_pair(o_ps, h, es[5], 5))
                        work_q.append(adds_pair(a2, a2g, es[6], es[7]))
                        work_q.append(pv_pair(o_ps, h, es[6], 6))
                        work_q.append(pv_pair(o_ps, h, es[7], 7))
                        work_q.append(finish_vh(o_ps, rb_ps, a2, a2g, a3, h, nsl))
                y_sb = y_pool.tile([P, NB // P, DIM], F16, tag="y")
                for t in range(NB // P):
                    for ot in range(2):
                        work_q.append(
                            proj_half(nq, t, ot, y_sb, tail=(nq == NQ - 1))
                        )

            while work_q:
                work_q.popleft()()

    nc.compile()
    return nc


def make_in_maps(query, key, value, proj_w):
    scale = float(HD) ** -0.5
    wt_full = np.ascontiguousarray(proj_w.T.astype(np.float32))  # [in, out]
    in_maps = []
    for core in range(N_CORES):
        b, hg = divmod(core, N_CORES // B)
        sl = slice(hg * HG, (hg + 1) * HG)
        # per-head partition-major views: [h][p][...]
        qt = (query[b].T[sl] * scale).astype(np.float16).reshape(2, P, N)
        kt = key[b].T[sl].astype(np.float16).reshape(2, P, S)
        # v[p, h, c, d] = value[c*128+p, h*128+d] -> [h][c][p][d]
        vv = (
            value[b][:, sl]
            .astype(np.float16)
            .reshape(SC, P, HEADS_PER_CORE, HD)
            .transpose(2, 0, 1, 3)
        )
        wt = wt_full[sl].astype(np.float16).reshape(2, P, DIM)

        pack = np.empty((P, PACK_COLS), dtype=np.float16)
        pack[:, OFF_QT0_B0:OFF_KT0] = qt[0, :, 0:NB]
        pack[:, OFF_KT0:OFF_V0_LO] = kt[0]
        pack[:, OFF_V0_LO:OFF_QT1_B0] = (
            vv[0, 0:8].transpose(1, 0, 2).reshape(P, 8 * HD)
        )
        pack[:, OFF_QT1_B0:OFF_KT1] = qt[1, :, 0:NB]
        pack[:, OFF_KT1:OFF_V0_HI] = kt[1]
        pack[:, OFF_V0_HI:OFF_V1] = vv[0, 8:16].transpose(1, 0, 2).reshape(P, 8 * HD)
        pack[:, OFF_V1:OFF_QT0_REST] = (
            vv[1].transpose(1, 0, 2).reshape(P, SC * HD)
        )
        pack[:, OFF_QT0_REST:OFF_QT1_REST] = qt[0, :, NB:]
        pack[:, OFF_QT1_REST:OFF_WT] = qt[1, :, NB:]
        pack[:, OFF_WT : OFF_WT + DIM] = wt[0]
        pack[:, OFF_WT + DIM :] = wt[1]
        in_maps.append({"inp": pack})
    return in_maps


def kernel(query, key, value, proj_w, proj_b):
    query = np.asarray(query)
    key = np.asarray(key)
    value = np.asarray(value)
    proj_w = np.asarray(proj_w)
    proj_b = np.asarray(proj_b)
    if "nc" not in _nc_cache:
        _nc_cache["nc"] = _build()
    nc = _nc_cache["nc"]

    in_maps = make_in_maps(query, key, value, proj_w)
    res = run_bass_kernel_spmd(nc, in_maps, list(range(N_CORES)))

    out = np.zeros((B, N, DIM), dtype=np.float32)
    for core in range(N_CORES):
        b = core // (N_CORES // B)
        # out dram is [p, chunk, o] with row n = chunk*128 + p
        part = res.results[core]["out"]
        out[b] += part.transpose(1, 0, 2).reshape(N, DIM).astype(np.float32)
    out += proj_b.astype(np.float32)
    return out
